# revision 33
# baseline (speedup 1.0000x reference)
"""Distributed Trainium2 kernel for the audio-visual contrastive loss.

Math (reference):
    a = l2norm(audio)  (B=32, Na=512, D=768)
    v = l2norm(visual) (B=32, Nv=256, D=768)
    token_sims[b,c,n,m] = (a[b,n] . v[c,m]) / T
    clip_sims = mean_n max_m token_sims          (B, B)
    loss = mean_b -0.5*(log_softmax(clip)[b,b] + log_softmax(clip.T)[b,b])

Distribution over 8 NeuronCores:
    - audio batch is sharded 4 clips/core; visual batch is sharded 4 clips/core
      for the (normalize + transpose) prep, then AllGather'd (as bf16, d-major)
      in 2 chunks so the second chunk's collective overlaps the first chunk's
      matmuls.
    - each core computes its (4, 32) block of clip_sims:
        S[n, m] = aT[:, n].T @ vT[:, m]  accumulated over 6 d-chunks in PSUM,
        row-max over m on VectorE, column-sum over n via a ones-matmul.
    - the (4,32) blocks are AllGather'd (tiny) and every core computes the
      final scalar loss redundantly.
"""

import os
import sys

for _p in ("/opt/trn_rl_repo",):
    if _p not in sys.path:
        sys.path.insert(0, _p)

import numpy as np

import concourse.bacc as bacc
import concourse.mybir as mybir
import concourse.tile as tile

N_CORES = 8
B = 32
NA = int(os.environ.get("KERNEL_NA", "512"))
NV = 256
D = int(os.environ.get("KERNEL_D", "768"))
TEMPERATURE = 0.1
BL = B // N_CORES            # 4 clips per core
AROWS = BL * NA              # 2048 audio rows per core
VROWS = BL * NV              # 1024 visual rows per core
KD = D // 128                # 6 contraction chunks
NT_A = AROWS // 128          # 16 audio row-tiles
NT_V = VROWS // 128          # 8 visual row-tiles
G = int(os.environ.get("KERNEL_GATHER_CHUNKS", "1"))  # visual AllGather chunks
VCH = VROWS // G             # visual rows per chunk per core
CPC = BL // G                # clips per chunk per core

F32 = mybir.dt.float32
BF16 = mybir.dt.bfloat16
AX = mybir.AxisListType
ALU = mybir.AluOpType
ACT = mybir.ActivationFunctionType


def build():
    nc = bacc.Bacc("TRN2", target_bir_lowering=False, debug=False,
                   num_devices=N_CORES)
    a_in = nc.declare_dram_parameter("audio", [AROWS, D], F32, isOutput=False)
    v_in = nc.declare_dram_parameter("visual", [VROWS, D], F32, isOutput=False)
    out = nc.declare_dram_parameter("out", [1, 1], F32, isOutput=True)
    ident_dram = nc.inline_tensor(np.eye(128, dtype=np.float32), name="ident")
    # vT/mx columns hold clips in "production order"
    #   cperm(c) = g*(8*CPC) + i*CPC + j  for global clip c = 4*i + g*CPC + j.
    # Row/column logsumexp sums are permutation-invariant; only the diagonal
    # extraction needs the map, via this permuted identity.
    pmask = np.zeros((32, 32), dtype=np.float32)
    for c in range(B):
        i, q = divmod(c, 4)
        g, j = divmod(q, CPC)
        pmask[c, g * (8 * CPC) + i * CPC + j] = 1.0
    pmask_dram = nc.inline_tensor(pmask, name="pmask")
    rg = [list(range(N_CORES))]

    with tile.TileContext(nc) as tc:
        with (
            tc.tile_pool(name="persist", bufs=1) as pp,
            tc.tile_pool(name="work", bufs=3) as wp,
            tc.tile_pool(name="ps", bufs=8, space="PSUM") as ps,
            tc.tile_pool(name="dram", bufs=1, space="DRAM") as dp,
        ):
            # ---- constants -------------------------------------------------
            ident_f32 = pp.tile([128, 128], F32, tag="identf")
            nc.sync.dma_start(out=ident_f32[:], in_=ident_dram[:])
            ident_bf = pp.tile([128, 128], BF16, tag="identb")
            nc.scalar.copy(ident_bf[:], ident_f32[:])
            ones = pp.tile([128, 1], F32, tag="ones")
            nc.gpsimd.memset(ones[:], 1.0)

            # ---- persistent tensors ---------------------------------------
            VTW = N_CORES * VROWS        # 8192 vT columns per d-chunk
            aT = [pp.tile([128, AROWS], BF16, tag=f"aT{k}", name=f"aT{k}")
                  for k in range(KD)]
            # single tiles so bounce/load DMAs can stride across d-chunks
            vstall = pp.tile([128, KD * VROWS], BF16, tag="vstall")
            vTall = pp.tile([128, KD * VTW], BF16, tag="vTall")
            mxw = (NA // 128) * 128
            mx = pp.tile([128, mxw], F32, tag="mx")

            # ---- row-tile prep: normalize rows, cast bf16, transpose ------
            # Row-tile prep, batched per-op so each engine runs bursts of the
            # same instruction instead of per-tile cross-engine round trips:
            #   wave of 8: DMA loads -> ACT Square(accum=ss) -> ACT sqrt ->
            #   DVE reciprocal -> ACT scaled casts -> PE transposes ->
            #   DVE psum->sbuf copies
            def prep_batch(src, t0, nb, dst_of):
                raws = []
                ssb = wp.tile([128, nb], F32, tag="ssb", name="ssb", bufs=2)
                for j in range(nb):
                    t = t0 + j
                    raw = wp.tile([128, D], F32, tag="raw", name="raw", bufs=8)
                    nc.sync.dma_start(out=raw[:],
                                      in_=src[t * 128:(t + 1) * 128, :])
                    sqs = wp.tile([128, D], F32, tag="sqs", name="sqs", bufs=2)
                    nc.scalar.activation(sqs[:], raw[:], ACT.Square,
                                         accum_out=ssb[:, j:j + 1])
                    raws.append(raw)
                nrm = wp.tile([128, nb], F32, tag="nrm", name="nrm", bufs=2)
                nc.scalar.sqrt(nrm[:], ssb[:])
                rnb = wp.tile([128, nb], F32, tag="rnb", name="rnb", bufs=2)
                nc.vector.reciprocal(rnb[:], nrm[:])
                for j in range(nb):
                    t = t0 + j
                    nbf = wp.tile([128, D], BF16, tag="nbf", name="nbf",
                                  bufs=4)
                    nc.scalar.activation(nbf[:], raws[j][:], ACT.Copy,
                                         bias=0.0, scale=rnb[:, j:j + 1])
                    for k in range(KD):
                        pt = ps.tile([128, 128], BF16, tag="ps", name="pt")
                        nc.tensor.transpose(pt[:],
                                            nbf[:, 128 * k:128 * (k + 1)],
                                            ident_bf[:])
                        dst_tile, col = dst_of(t, k)
                        nc.vector.tensor_copy(dst_tile[:, col:col + 128],
                                              pt[:])

            # ---- visual prep + bounce + chunked AllGather -----------------
            # DMA ring discipline (head-of-line blocking avoidance):
            #   nc.sync   : input loads only (never blocked by a semaphore)
            #   nc.gpsimd : bounce writes + collectives (SWDGE)
            #   nc.scalar : gathered-visual loads (qActDynamicHW; their AG
            #               waits land after all prep compute on ACT)
            # Gather buffers are f32-typed views (bitcast) of the bf16 data.
            nbv = max(2, NT_V // G)      # visual prep batch = one chunk
            vgath = []
            vst3 = vstall[:].rearrange("p (k c) -> p k c", k=KD)
            for g in range(G):
                for t0 in range(g * (NT_V // G), (g + 1) * (NT_V // G), nbv):
                    prep_batch(v_in, t0, nbv,
                               lambda t, k: (vstall, k * VROWS + t * 128))
                vb = dp.tile([KD, 128, VCH // 2], F32, tag=f"vb{g}",
                             name=f"vb{g}")
                nc.gpsimd.dma_start(
                    out=vb[:, :, :].rearrange("k p c -> p k c"),
                    in_=vst3[:, :, g * VCH:(g + 1) * VCH].bitcast(F32))
                vg = dp.tile([N_CORES * KD, 128, VCH // 2], F32, tag=f"vg{g}",
                             name=f"vg{g}", addr_space="Shared")
                nc.gpsimd.collective_compute(
                    "AllGather", ALU.bypass, replica_groups=rg,
                    ins=[vb[:, :, :].opt()], outs=[vg[:, :, :].opt()])
                vgath.append(vg)

            # ---- audio prep ----------------------------------------------
            for t0 in range(0, NT_A, 8):
                prep_batch(a_in, t0, min(8, NT_A - t0),
                           lambda t, k: (aT[k], t * 128))

            # ---- load gathered visual into SBUF ---------------------------
            # vTall col (within d-chunk k) = g*(8*VCH) + i*VCH + (j*256 + m)
            #   -> holds global clip c = 4*i + g*CPC + j at cperm position
            vT3 = vTall[:].rearrange("p (k c) -> p k c", k=KD)
            for g in range(G):
                for i in range(N_CORES):
                    cola = g * (N_CORES * VCH) + i * VCH
                    nc.scalar.dma_start(
                        out=vT3[:, :, cola:cola + VCH].bitcast(F32),
                        in_=vgath[g][i * KD:(i + 1) * KD].rearrange(
                            "k p c -> p k c"))

            # ---- main loop: S = aT.T @ vT, rowmax, accumulate -------------
            # mx col layout: nt*128 + b*32 + cperm
            for g in range(G):
                for b in range(BL):
                    for nt in range(NA // 128):
                        lcol = (b * (NA // 128) + nt) * 128
                        for h in range(CPC):
                            base = g * (N_CORES * VCH) + h * 2048
                            pss = [ps.tile([128, 512], F32, tag="ps",
                                           name="mm") for _ in range(4)]
                            for k in range(KD):
                                lhs = aT[k][:, lcol:lcol + 128]
                                for p in range(4):
                                    nc.tensor.matmul(
                                        pss[p][:], lhsT=lhs,
                                        rhs=vTall[:, k * VTW + base + p * 512:
                                                  k * VTW + base +
                                                  (p + 1) * 512],
                                        start=(k == 0), stop=(k == KD - 1))
                            for p in range(4):
                                c0 = g * (8 * CPC) + (h * 4 + p) * 2
                                mcol = nt * 128 + b * 32 + c0
                                nc.vector.tensor_reduce(
                                    out=mx[:, mcol:mcol + 2],
                                    in_=pss[p][:].rearrange(
                                        "p (j m) -> p j m", j=2),
                                    axis=AX.X, op=ALU.max)

            # ---- column sums of row-maxes: mean over n --------------------
            pclip = ps.tile([1, mxw], F32, tag="ps", name="pclip")
            nc.tensor.matmul(pclip[:], lhsT=ones[:], rhs=mx[:],
                             start=True, stop=True)
            csum = wp.tile([1, 128], F32, tag="csum")
            nc.vector.tensor_reduce(
                out=csum[:],
                in_=pclip[:].rearrange("p (nt bc) -> p bc nt", nt=NA // 128),
                axis=AX.X, op=ALU.add)
            clip_blk = wp.tile([1, 128], F32, tag="clipblk")
            nc.scalar.mul(clip_blk[:], csum[:], 1.0 / (NA * TEMPERATURE))

            # ---- gather the (4,32) clip blocks ----------------------------
            cb = dp.tile([1, 128], F32, tag="cb", name="cb")
            nc.sync.dma_start(out=cb[:], in_=clip_blk[:])
            call = dp.tile([N_CORES, 128], F32, tag="call", name="call",
                           addr_space="Shared")
            nc.gpsimd.collective_compute(
                "AllGather", ALU.bypass, replica_groups=rg,
                ins=[cb[:, :].opt()], outs=[call[:, :].opt()])

            # ---- final loss (computed redundantly on every core) ----------
            clip_sb = wp.tile([32, 32], F32, tag="clip")
            nc.sync.dma_start(
                out=clip_sb[:],
                in_=call[:, :].rearrange("a (b c) -> (a b) c", b=4))
            pT = ps.tile([32, 32], F32, tag="ps", name="pT")
            nc.tensor.matmul(pT[:], lhsT=clip_sb[:], rhs=ident_f32[0:32, 0:32],
                             is_transpose=True)
            clipT = wp.tile([32, 32], F32, tag="clipT")
            nc.scalar.copy(clipT[:], pT[:])

            def lse_rows(x, nm_tag):
                # no max-stabilization: |clip| <= 1/T = 10, exp is safe in f32
                ex = wp.tile([32, 32], F32, tag=nm_tag + "ex", name="ex")
                es = wp.tile([32, 1], F32, tag=nm_tag + "es", name="es")
                nc.scalar.activation(ex[:], x[:], ACT.Exp, accum_out=es[:])
                lse = wp.tile([32, 1], F32, tag=nm_tag + "lse", name="lse")
                nc.scalar.activation(lse[:], es[:], ACT.Ln)
                return lse

            lse1 = lse_rows(clip_sb, "r")
            lse2 = lse_rows(clipT, "c")
            pmask_sb = wp.tile([32, 32], F32, tag="pmask")
            nc.sync.dma_start(out=pmask_sb[:], in_=pmask_dram[:])
            dsc = wp.tile([32, 32], F32, tag="dsc")
            diag = wp.tile([32, 1], F32, tag="diag")
            nc.vector.tensor_mul(dsc[:], clip_sb[:], pmask_sb[:])
            nc.vector.reduce_sum(out=diag[:], in_=dsc[:], axis=AX.X)
            s = wp.tile([32, 1], F32, tag="s")
            nc.vector.tensor_add(s[:], lse1[:], lse2[:])
            lb = wp.tile([32, 1], F32, tag="lb")
            nc.vector.scalar_tensor_tensor(
                out=lb[:], in0=s[:], scalar=0.5, in1=diag[:],
                op0=ALU.mult, op1=ALU.subtract)
            pl = ps.tile([1, 1], F32, tag="ps", name="pl")
            nc.tensor.matmul(pl[:], lhsT=ones[0:32, :], rhs=lb[:],
                             start=True, stop=True)
            res = wp.tile([1, 1], F32, tag="res")
            nc.scalar.mul(res[:], pl[:], 1.0 / B)
            nc.sync.dma_start(out=out[:], in_=res[:])

    nc.finalize()
    return nc


_NC_CACHE = None


def kernel(audio_feats: np.ndarray, visual_feats: np.ndarray) -> np.ndarray:
    from concourse.bass_utils import run_bass_kernel_spmd

    global _NC_CACHE
    if _NC_CACHE is None:
        _NC_CACHE = build()
    nc = _NC_CACHE

    audio = np.ascontiguousarray(audio_feats, dtype=np.float32)
    visual = np.ascontiguousarray(visual_feats, dtype=np.float32)
    in_maps = []
    for i in range(N_CORES):
        in_maps.append({
            "audio": audio[i * BL:(i + 1) * BL].reshape(AROWS, D),
            "visual": visual[i * BL:(i + 1) * BL].reshape(VROWS, D),
        })
    res = run_bass_kernel_spmd(nc, in_maps, core_ids=list(range(N_CORES)))
    val = res.results[0]["out"][0, 0]
    return np.asarray(val, dtype=np.float32)


if __name__ == "__main__":
    rng = np.random.default_rng(0)
    a = rng.standard_normal((B, NA, D)).astype(np.float32)
    v = rng.standard_normal((B, NV, D)).astype(np.float32)
    print(kernel(a, v))


# revision 36
# speedup vs baseline: 1.0047x; 1.0047x over previous
"""Distributed Trainium2 kernel for the audio-visual contrastive loss.

Math (reference):
    a = l2norm(audio)  (B=32, Na=512, D=768)
    v = l2norm(visual) (B=32, Nv=256, D=768)
    token_sims[b,c,n,m] = (a[b,n] . v[c,m]) / T
    clip_sims = mean_n max_m token_sims          (B, B)
    loss = mean_b -0.5*(log_softmax(clip)[b,b] + log_softmax(clip.T)[b,b])

Distribution over 8 NeuronCores:
    - audio batch is sharded 4 clips/core; visual batch is sharded 4 clips/core
      for the (normalize + transpose) prep, then AllGather'd (as bf16, d-major)
      in 2 chunks so the second chunk's collective overlaps the first chunk's
      matmuls.
    - each core computes its (4, 32) block of clip_sims:
        S[n, m] = aT[:, n].T @ vT[:, m]  accumulated over 6 d-chunks in PSUM,
        row-max over m on VectorE, column-sum over n via a ones-matmul.
    - the (4,32) blocks are AllGather'd (tiny) and every core computes the
      final scalar loss redundantly.
"""

import os
import sys

for _p in ("/opt/trn_rl_repo",):
    if _p not in sys.path:
        sys.path.insert(0, _p)

import numpy as np

import concourse.bacc as bacc
import concourse.mybir as mybir
import concourse.tile as tile
from concourse.tile_rust import add_dep_helper

N_CORES = 8
B = 32
NA = int(os.environ.get("KERNEL_NA", "512"))
NV = 256
D = int(os.environ.get("KERNEL_D", "768"))
TEMPERATURE = 0.1
BL = B // N_CORES            # 4 clips per core
AROWS = BL * NA              # 2048 audio rows per core
VROWS = BL * NV              # 1024 visual rows per core
KD = D // 128                # 6 contraction chunks
NT_A = AROWS // 128          # 16 audio row-tiles
NT_V = VROWS // 128          # 8 visual row-tiles
G = int(os.environ.get("KERNEL_GATHER_CHUNKS", "1"))  # visual AllGather chunks
VCH = VROWS // G             # visual rows per chunk per core
CPC = BL // G                # clips per chunk per core

F32 = mybir.dt.float32
BF16 = mybir.dt.bfloat16
AX = mybir.AxisListType
ALU = mybir.AluOpType
ACT = mybir.ActivationFunctionType


def build():
    nc = bacc.Bacc("TRN2", target_bir_lowering=False, debug=False,
                   num_devices=N_CORES)
    a_in = nc.declare_dram_parameter("audio", [AROWS, D], F32, isOutput=False)
    v_in = nc.declare_dram_parameter("visual", [VROWS, D], F32, isOutput=False)
    out = nc.declare_dram_parameter("out", [1, 1], F32, isOutput=True)
    ident_dram = nc.inline_tensor(np.eye(128, dtype=np.float32), name="ident")
    # vT/mx columns hold clips in "production order"
    #   cperm(c) = g*(8*CPC) + i*CPC + j  for global clip c = 4*i + g*CPC + j.
    # Row/column logsumexp sums are permutation-invariant; only the diagonal
    # extraction needs the map, via this permuted identity.
    pmask = np.zeros((32, 32), dtype=np.float32)
    for c in range(B):
        i, q = divmod(c, 4)
        g, j = divmod(q, CPC)
        pmask[c, g * (8 * CPC) + i * CPC + j] = 1.0
    pmask_dram = nc.inline_tensor(pmask, name="pmask")
    rg = [list(range(N_CORES))]

    with tile.TileContext(nc) as tc:
        with (
            tc.tile_pool(name="persist", bufs=1) as pp,
            tc.tile_pool(name="work", bufs=3) as wp,
            tc.tile_pool(name="ps", bufs=8, space="PSUM") as ps,
            tc.tile_pool(name="dram", bufs=1, space="DRAM") as dp,
        ):
            # ---- constants -------------------------------------------------
            ident_f32 = pp.tile([128, 128], F32, tag="identf")
            nc.sync.dma_start(out=ident_f32[:], in_=ident_dram[:])
            ident_bf = pp.tile([128, 128], BF16, tag="identb")
            nc.scalar.copy(ident_bf[:], ident_f32[:])
            ones = pp.tile([128, 1], F32, tag="ones")
            nc.gpsimd.memset(ones[:], 1.0)

            # ---- persistent tensors ---------------------------------------
            VTW = N_CORES * VROWS        # 8192 vT columns per d-chunk
            aT = [pp.tile([128, AROWS], BF16, tag=f"aT{k}", name=f"aT{k}")
                  for k in range(KD)]
            # single tiles so bounce/load DMAs can stride across d-chunks
            vstall = pp.tile([128, KD * VROWS], BF16, tag="vstall")
            vTall = pp.tile([128, KD * VTW], BF16, tag="vTall")
            mxw = (NA // 128) * 128
            mx = pp.tile([128, mxw], F32, tag="mx")

            # ---- row-tile prep: normalize rows, cast bf16, transpose ------
            # Row-tile prep, batched per-op so each engine runs bursts of the
            # same instruction instead of per-tile cross-engine round trips:
            #   wave of 8: DMA loads -> ACT Square(accum=ss) -> ACT sqrt ->
            #   DVE reciprocal -> ACT scaled casts -> PE transposes ->
            #   DVE psum->sbuf copies
            def prep_batch(src, t0, nb, dst_of, load_group):
                raws = []
                ssb = wp.tile([128, nb], F32, tag="ssb", name="ssb", bufs=2)
                for j in range(nb):
                    t = t0 + j
                    raw = wp.tile([128, D], F32, tag="raw", name="raw", bufs=8)
                    load_group.append(
                        nc.sync.dma_start(out=raw[:],
                                          in_=src[t * 128:(t + 1) * 128, :]))
                    sqs = wp.tile([128, D], F32, tag="sqs", name="sqs", bufs=2)
                    nc.scalar.activation(sqs[:], raw[:], ACT.Square,
                                         accum_out=ssb[:, j:j + 1])
                    raws.append(raw)
                nrm = wp.tile([128, nb], F32, tag="nrm", name="nrm", bufs=2)
                nc.scalar.sqrt(nrm[:], ssb[:])
                rnb = wp.tile([128, nb], F32, tag="rnb", name="rnb", bufs=2)
                nc.vector.reciprocal(rnb[:], nrm[:])
                for j in range(nb):
                    t = t0 + j
                    nbf = wp.tile([128, D], BF16, tag="nbf", name="nbf",
                                  bufs=4)
                    nc.scalar.activation(nbf[:], raws[j][:], ACT.Copy,
                                         bias=0.0, scale=rnb[:, j:j + 1])
                    for k in range(KD):
                        pt = ps.tile([128, 128], BF16, tag="ps", name="pt")
                        nc.tensor.transpose(pt[:],
                                            nbf[:, 128 * k:128 * (k + 1)],
                                            ident_bf[:])
                        dst_tile, col = dst_of(t, k)
                        nc.vector.tensor_copy(dst_tile[:, col:col + 128],
                                              pt[:])

            # ---- visual prep + bounce + chunked AllGather -----------------
            # DMA ring discipline (head-of-line blocking avoidance):
            #   nc.sync   : input loads only (never blocked by a semaphore)
            #   nc.gpsimd : bounce writes + collectives (SWDGE)
            #   nc.scalar : gathered-visual loads (qActDynamicHW; their AG
            #               waits land after all prep compute on ACT)
            # Gather buffers are f32-typed views (bitcast) of the bf16 data.
            nbv = max(2, NT_V // G)      # visual prep batch = one chunk
            vis_loads, aud_loads1, aud_loads2 = [], [], []
            bounces, vt_loads = [], []
            vgath = []
            vst3 = vstall[:].rearrange("p (k c) -> p k c", k=KD)
            for g in range(G):
                for t0 in range(g * (NT_V // G), (g + 1) * (NT_V // G), nbv):
                    prep_batch(v_in, t0, nbv,
                               lambda t, k: (vstall, k * VROWS + t * 128),
                               vis_loads)
                vb = dp.tile([KD, 128, VCH // 2], F32, tag=f"vb{g}",
                             name=f"vb{g}")
                bounces.append(nc.sync.dma_start(
                    out=vb[:, :, :].rearrange("k p c -> p k c"),
                    in_=vst3[:, :, g * VCH:(g + 1) * VCH].bitcast(F32)))
                vg = dp.tile([N_CORES * KD, 128, VCH // 2], F32, tag=f"vg{g}",
                             name=f"vg{g}", addr_space="Shared")
                nc.gpsimd.collective_compute(
                    "AllGather", ALU.bypass, replica_groups=rg,
                    ins=[vb[:, :, :].opt()], outs=[vg[:, :, :].opt()])
                vgath.append(vg)

            # ---- audio prep ----------------------------------------------
            for t0 in range(0, NT_A, 8):
                prep_batch(a_in, t0, min(8, NT_A - t0),
                           lambda t, k: (aT[k], t * 128),
                           aud_loads1 if t0 == 0 else aud_loads2)

            # ---- load gathered visual into SBUF ---------------------------
            # vTall col (within d-chunk k) = g*(8*VCH) + i*VCH + (j*256 + m)
            #   -> holds global clip c = 4*i + g*CPC + j at cperm position
            vT3 = vTall[:].rearrange("p (k c) -> p k c", k=KD)
            for g in range(G):
                for i in range(N_CORES):
                    cola = g * (N_CORES * VCH) + i * VCH
                    vt_loads.append(nc.sync.dma_start(
                        out=vT3[:, :, cola:cola + VCH].bitcast(F32),
                        in_=vgath[g][i * KD:(i + 1) * KD].rearrange(
                            "k p c -> p k c")))

            # Explicit sync-ring ordering: the HWDGE ring is FIFO per engine,
            # and a DMA whose wait isn't met blocks everything behind it.
            # Keep never-blocked input loads ahead of semaphore-gated loads.
            ring_groups = [
                vis_loads,
                bounces[:max(1, G // 2)],
                aud_loads1,
                bounces[max(1, G // 2):],
                aud_loads2,
                vt_loads,
            ]
            prev = None
            for grp in ring_groups:
                if not grp:
                    continue
                if prev is not None:
                    for h in grp:
                        add_dep_helper(h.ins, prev.ins, sync=False,
                                       reason="sync-ring class order")
                prev = grp[-1]

            # ---- main loop: S = aT.T @ vT, rowmax, accumulate -------------
            # mx col layout: nt*128 + b*32 + cperm
            for g in range(G):
                for b in range(BL):
                    for nt in range(NA // 128):
                        lcol = (b * (NA // 128) + nt) * 128
                        for h in range(CPC):
                            base = g * (N_CORES * VCH) + h * 2048
                            pss = [ps.tile([128, 512], F32, tag="ps",
                                           name="mm") for _ in range(4)]
                            for k in range(KD):
                                lhs = aT[k][:, lcol:lcol + 128]
                                for p in range(4):
                                    nc.tensor.matmul(
                                        pss[p][:], lhsT=lhs,
                                        rhs=vTall[:, k * VTW + base + p * 512:
                                                  k * VTW + base +
                                                  (p + 1) * 512],
                                        start=(k == 0), stop=(k == KD - 1))
                            for p in range(4):
                                c0 = g * (8 * CPC) + (h * 4 + p) * 2
                                mcol = nt * 128 + b * 32 + c0
                                nc.vector.tensor_reduce(
                                    out=mx[:, mcol:mcol + 2],
                                    in_=pss[p][:].rearrange(
                                        "p (j m) -> p j m", j=2),
                                    axis=AX.X, op=ALU.max)

            # ---- column sums of row-maxes: mean over n --------------------
            pclip = ps.tile([1, mxw], F32, tag="ps", name="pclip")
            nc.tensor.matmul(pclip[:], lhsT=ones[:], rhs=mx[:],
                             start=True, stop=True)
            csum = wp.tile([1, 128], F32, tag="csum")
            nc.vector.tensor_reduce(
                out=csum[:],
                in_=pclip[:].rearrange("p (nt bc) -> p bc nt", nt=NA // 128),
                axis=AX.X, op=ALU.add)
            clip_blk = wp.tile([1, 128], F32, tag="clipblk")
            nc.scalar.mul(clip_blk[:], csum[:], 1.0 / (NA * TEMPERATURE))

            # ---- gather the (4,32) clip blocks ----------------------------
            cb = dp.tile([1, 128], F32, tag="cb", name="cb")
            nc.sync.dma_start(out=cb[:], in_=clip_blk[:])
            call = dp.tile([N_CORES, 128], F32, tag="call", name="call",
                           addr_space="Shared")
            nc.gpsimd.collective_compute(
                "AllGather", ALU.bypass, replica_groups=rg,
                ins=[cb[:, :].opt()], outs=[call[:, :].opt()])

            # ---- final loss (computed redundantly on every core) ----------
            clip_sb = wp.tile([32, 32], F32, tag="clip")
            nc.sync.dma_start(
                out=clip_sb[:],
                in_=call[:, :].rearrange("a (b c) -> (a b) c", b=4))
            pT = ps.tile([32, 32], F32, tag="ps", name="pT")
            nc.tensor.matmul(pT[:], lhsT=clip_sb[:], rhs=ident_f32[0:32, 0:32],
                             is_transpose=True)
            clipT = wp.tile([32, 32], F32, tag="clipT")
            nc.scalar.copy(clipT[:], pT[:])

            def lse_rows(x, nm_tag):
                # no max-stabilization: |clip| <= 1/T = 10, exp is safe in f32
                ex = wp.tile([32, 32], F32, tag=nm_tag + "ex", name="ex")
                es = wp.tile([32, 1], F32, tag=nm_tag + "es", name="es")
                nc.scalar.activation(ex[:], x[:], ACT.Exp, accum_out=es[:])
                lse = wp.tile([32, 1], F32, tag=nm_tag + "lse", name="lse")
                nc.scalar.activation(lse[:], es[:], ACT.Ln)
                return lse

            lse1 = lse_rows(clip_sb, "r")
            lse2 = lse_rows(clipT, "c")
            pmask_sb = wp.tile([32, 32], F32, tag="pmask")
            nc.sync.dma_start(out=pmask_sb[:], in_=pmask_dram[:])
            dsc = wp.tile([32, 32], F32, tag="dsc")
            diag = wp.tile([32, 1], F32, tag="diag")
            nc.vector.tensor_mul(dsc[:], clip_sb[:], pmask_sb[:])
            nc.vector.reduce_sum(out=diag[:], in_=dsc[:], axis=AX.X)
            s = wp.tile([32, 1], F32, tag="s")
            nc.vector.tensor_add(s[:], lse1[:], lse2[:])
            lb = wp.tile([32, 1], F32, tag="lb")
            nc.vector.scalar_tensor_tensor(
                out=lb[:], in0=s[:], scalar=0.5, in1=diag[:],
                op0=ALU.mult, op1=ALU.subtract)
            pl = ps.tile([1, 1], F32, tag="ps", name="pl")
            nc.tensor.matmul(pl[:], lhsT=ones[0:32, :], rhs=lb[:],
                             start=True, stop=True)
            res = wp.tile([1, 1], F32, tag="res")
            nc.scalar.mul(res[:], pl[:], 1.0 / B)
            nc.sync.dma_start(out=out[:], in_=res[:])

    nc.finalize()
    return nc


_NC_CACHE = None


def kernel(audio_feats: np.ndarray, visual_feats: np.ndarray) -> np.ndarray:
    from concourse.bass_utils import run_bass_kernel_spmd

    global _NC_CACHE
    if _NC_CACHE is None:
        _NC_CACHE = build()
    nc = _NC_CACHE

    audio = np.ascontiguousarray(audio_feats, dtype=np.float32)
    visual = np.ascontiguousarray(visual_feats, dtype=np.float32)
    in_maps = []
    for i in range(N_CORES):
        in_maps.append({
            "audio": audio[i * BL:(i + 1) * BL].reshape(AROWS, D),
            "visual": visual[i * BL:(i + 1) * BL].reshape(VROWS, D),
        })
    res = run_bass_kernel_spmd(nc, in_maps, core_ids=list(range(N_CORES)))
    val = res.results[0]["out"][0, 0]
    return np.asarray(val, dtype=np.float32)


if __name__ == "__main__":
    rng = np.random.default_rng(0)
    a = rng.standard_normal((B, NA, D)).astype(np.float32)
    v = rng.standard_normal((B, NV, D)).astype(np.float32)
    print(kernel(a, v))


# revision 40
# speedup vs baseline: 1.1339x; 1.1285x over previous
"""Distributed Trainium2 kernel for the audio-visual contrastive loss.

Math (reference):
    a = l2norm(audio)  (B=32, Na=512, D=768)
    v = l2norm(visual) (B=32, Nv=256, D=768)
    token_sims[b,c,n,m] = (a[b,n] . v[c,m]) / T
    clip_sims = mean_n max_m token_sims          (B, B)
    loss = mean_b -0.5*(log_softmax(clip)[b,b] + log_softmax(clip.T)[b,b])

Distribution over 8 NeuronCores:
    - audio batch is sharded 4 clips/core; visual batch is sharded 4 clips/core
      for the (normalize + transpose) prep, then AllGather'd (as bf16, d-major)
      in 2 chunks so the second chunk's collective overlaps the first chunk's
      matmuls.
    - each core computes its (4, 32) block of clip_sims:
        S[n, m] = aT[:, n].T @ vT[:, m]  accumulated over 6 d-chunks in PSUM,
        row-max over m on VectorE, column-sum over n via a ones-matmul.
    - the (4,32) blocks are AllGather'd (tiny) and every core computes the
      final scalar loss redundantly.
"""

import os
import sys

for _p in ("/opt/trn_rl_repo",):
    if _p not in sys.path:
        sys.path.insert(0, _p)

import numpy as np

import concourse.bacc as bacc
import concourse.mybir as mybir
import concourse.tile as tile
from concourse.tile_rust import add_dep_helper

N_CORES = 8
B = 32
NA = int(os.environ.get("KERNEL_NA", "512"))
NV = 256
D = int(os.environ.get("KERNEL_D", "768"))
TEMPERATURE = 0.1
BL = B // N_CORES            # 4 clips per core
AROWS = BL * NA              # 2048 audio rows per core
VROWS = BL * NV              # 1024 visual rows per core
KD = D // 128                # 6 contraction chunks
NT_A = AROWS // 128          # 16 audio row-tiles
NT_V = VROWS // 128          # 8 visual row-tiles
G = int(os.environ.get("KERNEL_GATHER_CHUNKS", "1"))  # visual AllGather chunks
VCH = VROWS // G             # visual rows per chunk per core
CPC = BL // G                # clips per chunk per core

F32 = mybir.dt.float32
BF16 = mybir.dt.bfloat16
AX = mybir.AxisListType
ALU = mybir.AluOpType
ACT = mybir.ActivationFunctionType


def build():
    nc = bacc.Bacc("TRN2", target_bir_lowering=False, debug=False,
                   num_devices=N_CORES)
    a_in = nc.declare_dram_parameter("audio", [AROWS, D], F32, isOutput=False)
    v_in = nc.declare_dram_parameter("visual", [VROWS, D], F32, isOutput=False)
    out = nc.declare_dram_parameter("out", [1, 1], F32, isOutput=True)
    ident_dram = nc.inline_tensor(np.eye(128, dtype=np.float32), name="ident")
    # vT/mx columns hold clips in "production order"
    #   cperm(c) = g*(8*CPC) + i*CPC + j  for global clip c = 4*i + g*CPC + j.
    # Row/column logsumexp sums are permutation-invariant; only the diagonal
    # extraction needs the map, via this permuted identity.
    pmask = np.zeros((32, 32), dtype=np.float32)
    for c in range(B):
        i, q = divmod(c, 4)
        g, j = divmod(q, CPC)
        pmask[c, g * (8 * CPC) + i * CPC + j] = 1.0
    pmask_dram = nc.inline_tensor(pmask, name="pmask")
    rg = [list(range(N_CORES))]

    with tile.TileContext(nc) as tc:
        with (
            tc.tile_pool(name="persist", bufs=1) as pp,
            tc.tile_pool(name="work", bufs=3) as wp,
            tc.tile_pool(name="ps", bufs=8, space="PSUM") as ps,
            tc.tile_pool(name="dram", bufs=1, space="DRAM") as dp,
        ):
            # ---- constants -------------------------------------------------
            ident_f32 = pp.tile([128, 128], F32, tag="identf")
            nc.sync.dma_start(out=ident_f32[:], in_=ident_dram[:])
            ident_bf = pp.tile([128, 128], BF16, tag="identb")
            nc.scalar.copy(ident_bf[:], ident_f32[:])
            ones = pp.tile([128, 1], F32, tag="ones")
            nc.gpsimd.memset(ones[:], 1.0)

            # ---- persistent tensors ---------------------------------------
            VTW = N_CORES * VROWS        # 8192 vT columns per d-chunk
            aT = [pp.tile([128, AROWS], BF16, tag=f"aT{k}", name=f"aT{k}")
                  for k in range(KD)]
            # single tile so bounce DMAs can stride across d-chunks
            vstall = pp.tile([128, KD * VROWS], BF16, tag="vstall")
            # separate per-d-chunk gather destinations keep subtile
            # dependency tracking fine-grained for the matmul reads
            vT = [pp.tile([128, VTW], BF16, tag=f"vT{k}", name=f"vT{k}")
                  for k in range(KD)]
            mxw = (NA // 128) * 128
            mx = pp.tile([128, mxw], F32, tag="mx")

            # warmup collective: absorbs first-collective staging latency
            # while the input DMAs run
            wu_in = dp.tile([1, 32], F32, tag="wu_in", name="wu_in")
            wu_out = dp.tile([N_CORES, 32], F32, tag="wu_out", name="wu_out",
                             addr_space="Shared")
            wu_sb = pp.tile([1, 32], F32, tag="wu_sb")
            nc.gpsimd.memset(wu_sb[:], 0.0)
            nc.gpsimd.dma_start(out=wu_in[:], in_=wu_sb[:])
            nc.gpsimd.collective_compute(
                "AllGather", ALU.bypass, replica_groups=rg,
                ins=[wu_in[:, :].opt()], outs=[wu_out[:, :].opt()])

            # ---- row-tile prep: normalize rows, cast bf16, transpose ------
            # Row-tile prep, batched per-op so each engine runs bursts of the
            # same instruction instead of per-tile cross-engine round trips:
            #   wave of 8: DMA loads -> ACT Square(accum=ss) -> ACT sqrt ->
            #   DVE reciprocal -> ACT scaled casts -> PE transposes ->
            #   DVE psum->sbuf copies
            def prep_batch(src, t0, nb, dst_of, load_group):
                raws = []
                ssb = wp.tile([128, nb], F32, tag="ssb", name="ssb", bufs=2)
                for j in range(nb):
                    t = t0 + j
                    raw = wp.tile([128, D], F32, tag="raw", name="raw", bufs=8)
                    load_group.append(
                        nc.sync.dma_start(out=raw[:],
                                          in_=src[t * 128:(t + 1) * 128, :]))
                    sqs = wp.tile([128, D], F32, tag="sqs", name="sqs", bufs=2)
                    nc.scalar.activation(sqs[:], raw[:], ACT.Square,
                                         accum_out=ssb[:, j:j + 1])
                    raws.append(raw)
                nrm = wp.tile([128, nb], F32, tag="nrm", name="nrm", bufs=2)
                nc.scalar.sqrt(nrm[:], ssb[:])
                rnb = wp.tile([128, nb], F32, tag="rnb", name="rnb", bufs=2)
                nc.vector.reciprocal(rnb[:], nrm[:])
                for j in range(nb):
                    t = t0 + j
                    nbf = wp.tile([128, D], BF16, tag="nbf", name="nbf",
                                  bufs=4)
                    nc.scalar.activation(nbf[:], raws[j][:], ACT.Copy,
                                         bias=0.0, scale=rnb[:, j:j + 1])
                    for k in range(KD):
                        pt = ps.tile([128, 128], BF16, tag="ps", name="pt")
                        nc.tensor.transpose(pt[:],
                                            nbf[:, 128 * k:128 * (k + 1)],
                                            ident_bf[:])
                        dst_tile, col = dst_of(t, k)
                        nc.vector.tensor_copy(dst_tile[:, col:col + 128],
                                              pt[:])

            # ---- visual prep + bounce + chunked AllGather -----------------
            # DMA ring discipline (head-of-line blocking avoidance):
            #   nc.sync   : input loads only (never blocked by a semaphore)
            #   nc.gpsimd : bounce writes + collectives (SWDGE)
            #   nc.scalar : gathered-visual loads (qActDynamicHW; their AG
            #               waits land after all prep compute on ACT)
            # Gather buffers are f32-typed views (bitcast) of the bf16 data.
            nbv = max(2, NT_V // G)      # visual prep batch = one chunk
            vis_loads, aud_loads1, aud_loads2 = [], [], []
            bounces, vt_loads = [], []
            vgath = []
            vst3 = vstall[:].rearrange("p (k c) -> p k c", k=KD)
            for g in range(G):
                for t0 in range(g * (NT_V // G), (g + 1) * (NT_V // G), nbv):
                    prep_batch(v_in, t0, nbv,
                               lambda t, k: (vstall, k * VROWS + t * 128),
                               vis_loads)
                vb = dp.tile([KD, 128, VCH // 2], F32, tag=f"vb{g}",
                             name=f"vb{g}")
                # scalar (qAct) HWDGE ring: empty, so the bounce isn't queued
                # behind input loads; its short wait stalls ACT only briefly
                bounces.append(nc.scalar.dma_start(
                    out=vb[:, :, :].rearrange("k p c -> p k c"),
                    in_=vst3[:, :, g * VCH:(g + 1) * VCH].bitcast(F32)))
                vg = dp.tile([N_CORES * KD, 128, VCH // 2], F32, tag=f"vg{g}",
                             name=f"vg{g}", addr_space="Shared")
                nc.gpsimd.collective_compute(
                    "AllGather", ALU.bypass, replica_groups=rg,
                    ins=[vb[:, :, :].opt()], outs=[vg[:, :, :].opt()])
                vgath.append(vg)

            # ---- audio prep ----------------------------------------------
            for t0 in range(0, NT_A, 8):
                prep_batch(a_in, t0, min(8, NT_A - t0),
                           lambda t, k: (aT[k], t * 128),
                           aud_loads1 if t0 == 0 else aud_loads2)

            # ---- load gathered visual into SBUF ---------------------------
            # vTall col (within d-chunk k) = g*(8*VCH) + i*VCH + (j*256 + m)
            #   -> holds global clip c = 4*i + g*CPC + j at cperm position
            for g in range(G):
                for i in range(N_CORES):
                    cola = g * (N_CORES * VCH) + i * VCH
                    for k in range(KD):
                        vt_loads.append(nc.sync.dma_start(
                            out=vT[k][:, cola:cola + VCH].bitcast(F32),
                            in_=vgath[g][i * KD + k]))

            # Explicit sync-ring ordering: the HWDGE ring is FIFO per engine,
            # and a DMA whose wait isn't met blocks everything behind it.
            # Keep never-blocked input loads ahead of semaphore-gated loads.
            ring_groups = [
                vis_loads,
                aud_loads1,
                aud_loads2,
                vt_loads,
            ]
            prev = None
            for grp in ring_groups:
                if not grp:
                    continue
                if prev is not None:
                    for h in grp:
                        add_dep_helper(h.ins, prev.ins, sync=False,
                                       reason="sync-ring class order")
                prev = grp[-1]

            # ---- main loop: S = aT.T @ vT, rowmax, accumulate -------------
            # mx col layout: nt*128 + b*32 + cperm
            for g in range(G):
                for b in range(BL):
                    for nt in range(NA // 128):
                        lcol = (b * (NA // 128) + nt) * 128
                        for h in range(CPC):
                            base = g * (N_CORES * VCH) + h * 2048
                            pss = [ps.tile([128, 512], F32, tag="ps",
                                           name="mm") for _ in range(4)]
                            for k in range(KD):
                                lhs = aT[k][:, lcol:lcol + 128]
                                for p in range(4):
                                    nc.tensor.matmul(
                                        pss[p][:], lhsT=lhs,
                                        rhs=vT[k][:, base + p * 512:
                                                  base + (p + 1) * 512],
                                        start=(k == 0), stop=(k == KD - 1))
                            for p in range(4):
                                c0 = g * (8 * CPC) + (h * 4 + p) * 2
                                mcol = nt * 128 + b * 32 + c0
                                nc.vector.tensor_reduce(
                                    out=mx[:, mcol:mcol + 2],
                                    in_=pss[p][:].rearrange(
                                        "p (j m) -> p j m", j=2),
                                    axis=AX.X, op=ALU.max)

            # ---- column sums of row-maxes: mean over n --------------------
            pclip = ps.tile([1, mxw], F32, tag="ps", name="pclip")
            nc.tensor.matmul(pclip[:], lhsT=ones[:], rhs=mx[:],
                             start=True, stop=True)
            csum = wp.tile([1, 128], F32, tag="csum")
            nc.vector.tensor_reduce(
                out=csum[:],
                in_=pclip[:].rearrange("p (nt bc) -> p bc nt", nt=NA // 128),
                axis=AX.X, op=ALU.add)
            clip_blk = wp.tile([1, 128], F32, tag="clipblk")
            nc.scalar.mul(clip_blk[:], csum[:], 1.0 / (NA * TEMPERATURE))

            # ---- gather the (4,32) clip blocks ----------------------------
            cb = dp.tile([1, 128], F32, tag="cb", name="cb")
            nc.sync.dma_start(out=cb[:], in_=clip_blk[:])
            call = dp.tile([N_CORES, 128], F32, tag="call", name="call",
                           addr_space="Shared")
            nc.gpsimd.collective_compute(
                "AllGather", ALU.bypass, replica_groups=rg,
                ins=[cb[:, :].opt()], outs=[call[:, :].opt()])

            # ---- final loss (computed redundantly on every core) ----------
            clip_sb = wp.tile([32, 32], F32, tag="clip")
            nc.sync.dma_start(
                out=clip_sb[:],
                in_=call[:, :].rearrange("a (b c) -> (a b) c", b=4))
            pT = ps.tile([32, 32], F32, tag="ps", name="pT")
            nc.tensor.matmul(pT[:], lhsT=clip_sb[:], rhs=ident_f32[0:32, 0:32],
                             is_transpose=True)
            clipT = wp.tile([32, 32], F32, tag="clipT")
            nc.scalar.copy(clipT[:], pT[:])

            def lse_rows(x, nm_tag):
                # no max-stabilization: |clip| <= 1/T = 10, exp is safe in f32
                ex = wp.tile([32, 32], F32, tag=nm_tag + "ex", name="ex")
                es = wp.tile([32, 1], F32, tag=nm_tag + "es", name="es")
                nc.scalar.activation(ex[:], x[:], ACT.Exp, accum_out=es[:])
                lse = wp.tile([32, 1], F32, tag=nm_tag + "lse", name="lse")
                nc.scalar.activation(lse[:], es[:], ACT.Ln)
                return lse

            lse1 = lse_rows(clip_sb, "r")
            lse2 = lse_rows(clipT, "c")
            pmask_sb = wp.tile([32, 32], F32, tag="pmask")
            nc.sync.dma_start(out=pmask_sb[:], in_=pmask_dram[:])
            dsc = wp.tile([32, 32], F32, tag="dsc")
            diag = wp.tile([32, 1], F32, tag="diag")
            nc.vector.tensor_mul(dsc[:], clip_sb[:], pmask_sb[:])
            nc.vector.reduce_sum(out=diag[:], in_=dsc[:], axis=AX.X)
            s = wp.tile([32, 1], F32, tag="s")
            nc.vector.tensor_add(s[:], lse1[:], lse2[:])
            lb = wp.tile([32, 1], F32, tag="lb")
            nc.vector.scalar_tensor_tensor(
                out=lb[:], in0=s[:], scalar=0.5, in1=diag[:],
                op0=ALU.mult, op1=ALU.subtract)
            pl = ps.tile([1, 1], F32, tag="ps", name="pl")
            nc.tensor.matmul(pl[:], lhsT=ones[0:32, :], rhs=lb[:],
                             start=True, stop=True)
            res = wp.tile([1, 1], F32, tag="res")
            nc.scalar.mul(res[:], pl[:], 1.0 / B)
            nc.sync.dma_start(out=out[:], in_=res[:])

    nc.finalize()
    return nc


_NC_CACHE = None


def kernel(audio_feats: np.ndarray, visual_feats: np.ndarray) -> np.ndarray:
    from concourse.bass_utils import run_bass_kernel_spmd

    global _NC_CACHE
    if _NC_CACHE is None:
        _NC_CACHE = build()
    nc = _NC_CACHE

    audio = np.ascontiguousarray(audio_feats, dtype=np.float32)
    visual = np.ascontiguousarray(visual_feats, dtype=np.float32)
    in_maps = []
    for i in range(N_CORES):
        in_maps.append({
            "audio": audio[i * BL:(i + 1) * BL].reshape(AROWS, D),
            "visual": visual[i * BL:(i + 1) * BL].reshape(VROWS, D),
        })
    res = run_bass_kernel_spmd(nc, in_maps, core_ids=list(range(N_CORES)))
    val = res.results[0]["out"][0, 0]
    return np.asarray(val, dtype=np.float32)


if __name__ == "__main__":
    rng = np.random.default_rng(0)
    a = rng.standard_normal((B, NA, D)).astype(np.float32)
    v = rng.standard_normal((B, NV, D)).astype(np.float32)
    print(kernel(a, v))


# revision 48
# speedup vs baseline: 1.4366x; 1.2670x over previous
"""Distributed Trainium2 kernel for the audio-visual contrastive loss.

Math (reference):
    a = l2norm(audio)  (B=32, Na=512, D=768)
    v = l2norm(visual) (B=32, Nv=256, D=768)
    token_sims[b,c,n,m] = (a[b,n] . v[c,m]) / T
    clip_sims = mean_n max_m token_sims          (B, B)
    loss = mean_b -0.5*(log_softmax(clip)[b,b] + log_softmax(clip.T)[b,b])

Distribution over 8 NeuronCores:
    - audio batch is sharded 4 clips/core; visual batch is sharded 4 clips/core
      for the (normalize + transpose) prep, then AllGather'd (as bf16, d-major)
      in 2 chunks so the second chunk's collective overlaps the first chunk's
      matmuls.
    - each core computes its (4, 32) block of clip_sims:
        S[n, m] = aT[:, n].T @ vT[:, m]  accumulated over 6 d-chunks in PSUM,
        row-max over m on VectorE, column-sum over n via a ones-matmul.
    - the (4,32) blocks are AllGather'd (tiny) and every core computes the
      final scalar loss redundantly.
"""

import os
import sys

for _p in ("/opt/trn_rl_repo",):
    if _p not in sys.path:
        sys.path.insert(0, _p)

import numpy as np

import concourse.bacc as bacc
import concourse.mybir as mybir
import concourse.tile as tile
from concourse.tile_rust import add_dep_helper

N_CORES = 8
B = 32
NA = int(os.environ.get("KERNEL_NA", "512"))
NV = 256
D = int(os.environ.get("KERNEL_D", "768"))
TEMPERATURE = 0.1
BL = B // N_CORES            # 4 clips per core
AROWS = BL * NA              # 2048 audio rows per core
VROWS = BL * NV              # 1024 visual rows per core
KD = D // 128                # 6 contraction chunks
NT_A = AROWS // 128          # 16 audio row-tiles
NT_V = VROWS // 128          # 8 visual row-tiles
G = int(os.environ.get("KERNEL_GATHER_CHUNKS", "1"))  # visual AllGather chunks
VCH = VROWS // G             # visual rows per chunk per core
CPC = BL // G                # clips per chunk per core

F32 = mybir.dt.float32
BF16 = mybir.dt.bfloat16
FP8 = mybir.dt.float8e4
AX = mybir.AxisListType
ALU = mybir.AluOpType
ACT = mybir.ActivationFunctionType
SCL = 16.0                   # fp8 pre-scale (folded into the norm rsqrt)


def build():
    nc = bacc.Bacc("TRN2", target_bir_lowering=False, debug=False,
                   num_devices=N_CORES)
    a_in = nc.declare_dram_parameter("audio", [AROWS, D], F32, isOutput=False)
    v_in = nc.declare_dram_parameter("visual", [VROWS, D], F32, isOutput=False)
    out = nc.declare_dram_parameter("out", [1, 1], F32, isOutput=True)
    ident_dram = nc.inline_tensor(np.eye(128, dtype=np.float32), name="ident")
    # vT/mx columns hold clips in "production order"
    #   cperm(c) = g*(8*CPC) + i*CPC + j  for global clip c = 4*i + g*CPC + j.
    # Row/column logsumexp sums are permutation-invariant; only the diagonal
    # extraction needs the map, via this permuted identity.
    pmask = np.zeros((32, 32), dtype=np.float32)
    for c in range(B):
        i, q = divmod(c, 4)
        g, j = divmod(q, CPC)
        pmask[c, g * (8 * CPC) + i * CPC + j] = 1.0
    pmask_dram = nc.inline_tensor(pmask, name="pmask")
    rg = [list(range(N_CORES))]

    with tile.TileContext(nc) as tc:
        with (
            tc.tile_pool(name="persist", bufs=1) as pp,
            tc.tile_pool(name="work", bufs=3) as wp,
            tc.tile_pool(name="ps", bufs=8, space="PSUM") as ps,
            tc.tile_pool(name="dram", bufs=1, space="DRAM") as dp,
        ):
            # ---- constants -------------------------------------------------
            ident_f32 = pp.tile([128, 128], F32, tag="identf")
            nc.sync.dma_start(out=ident_f32[:], in_=ident_dram[:])
            ident_bf = pp.tile([128, 128], BF16, tag="identb")
            nc.scalar.copy(ident_bf[:], ident_f32[:])
            ones = pp.tile([128, 1], F32, tag="ones")
            nc.gpsimd.memset(ones[:], 1.0)

            # ---- persistent tensors ---------------------------------------
            # fp8 operands in DoubleRow ko-paired layout: tile k2 holds
            # d-chunk 2*k2 at ko=0 and 2*k2+1 at ko=1 (column offset AROWS/VTW)
            VTW = N_CORES * VROWS        # 8192 vT columns per d-chunk
            KD2 = KD // 2
            aTf = [pp.tile([128, 2 * AROWS], FP8, tag=f"aT8{k2}",
                           name=f"aT8{k2}") for k2 in range(KD2)]
            # single tile so bounce DMAs can stride across d-chunks
            vstall = pp.tile([128, KD * VROWS], FP8, tag="vstall")
            # separate per-(k2) gather destinations keep subtile dependency
            # tracking fine-grained for the matmul reads
            vTf = [pp.tile([128, 2 * VTW], FP8, tag=f"vT8{k2}",
                           name=f"vT8{k2}") for k2 in range(KD2)]
            mxw = (NA // 128) * 128
            mx = pp.tile([128, mxw], F32, tag="mx")

            # warmup collective: absorbs first-collective staging latency
            # while the input DMAs run
            wu_in = dp.tile([1, 32], F32, tag="wu_in", name="wu_in")
            wu_out = dp.tile([N_CORES, 32], F32, tag="wu_out", name="wu_out",
                             addr_space="Shared")
            wu_sb = pp.tile([1, 32], F32, tag="wu_sb")
            nc.gpsimd.memset(wu_sb[:], 0.0)
            nc.gpsimd.dma_start(out=wu_in[:], in_=wu_sb[:])
            nc.gpsimd.collective_compute(
                "AllGather", ALU.bypass, replica_groups=rg,
                ins=[wu_in[:, :].opt()], outs=[wu_out[:, :].opt()])

            # ---- row-tile prep: normalize rows, cast bf16, transpose ------
            # Row-tile prep, batched per-op so each engine runs bursts of the
            # same instruction instead of per-tile cross-engine round trips:
            #   wave of 8: DMA loads -> ACT Square(accum=ss) -> ACT sqrt ->
            #   DVE reciprocal -> ACT scaled casts -> PE transposes ->
            #   DVE psum->sbuf copies
            def prep_batch(src, t0, nb, dst_of, load_group):
                raws = []
                ssb = wp.tile([128, nb], F32, tag="ssb", name="ssb", bufs=2)
                for j in range(nb):
                    t = t0 + j
                    raw = wp.tile([128, D], F32, tag="raw", name="raw", bufs=8)
                    load_group.append(
                        nc.sync.dma_start(out=raw[:],
                                          in_=src[t * 128:(t + 1) * 128, :]))
                    sqs = wp.tile([128, D], F32, tag="sqs", name="sqs", bufs=2)
                    nc.scalar.activation(sqs[:], raw[:], ACT.Square,
                                         accum_out=ssb[:, j:j + 1])
                    raws.append(raw)
                nrm = wp.tile([128, nb], F32, tag="nrm", name="nrm", bufs=2)
                # norm/SCL, so 1/nrm scales rows by SCL/||x|| (fp8 pre-scale)
                nc.scalar.activation(nrm[:], ssb[:], ACT.Sqrt,
                                     scale=1.0 / (SCL * SCL))
                rnb = wp.tile([128, nb], F32, tag="rnb", name="rnb", bufs=2)
                nc.vector.reciprocal(rnb[:], nrm[:])
                for j in range(nb):
                    t = t0 + j
                    nbf = wp.tile([128, D], BF16, tag="nbf", name="nbf",
                                  bufs=4)
                    nc.scalar.activation(nbf[:], raws[j][:], ACT.Copy,
                                         bias=0.0, scale=rnb[:, j:j + 1])
                    for k in range(KD):
                        pt = ps.tile([128, 128], BF16, tag="ps", name="pt")
                        nc.tensor.transpose(pt[:],
                                            nbf[:, 128 * k:128 * (k + 1)],
                                            ident_bf[:])
                        dst_tile, col = dst_of(t, k)
                        nc.scalar.copy(dst_tile[:, col:col + 128], pt[:])

            # ---- visual prep + bounce + chunked AllGather -----------------
            # DMA ring discipline (head-of-line blocking avoidance):
            #   nc.sync   : input loads only (never blocked by a semaphore)
            #   nc.gpsimd : bounce writes + collectives (SWDGE)
            #   nc.scalar : gathered-visual loads (qActDynamicHW; their AG
            #               waits land after all prep compute on ACT)
            # Gather buffers are f32-typed views (bitcast) of the bf16 data.
            nbv = max(2, NT_V // G)      # visual prep batch = one chunk
            vis_loads, aud_loads1, aud_loads2 = [], [], []
            bounces, vt_loads = [], []
            vgath = []
            vst3 = vstall[:].rearrange("p (k c) -> p k c", k=KD)
            for g in range(G):
                for t0 in range(g * (NT_V // G), (g + 1) * (NT_V // G), nbv):
                    prep_batch(v_in, t0, nbv,
                               lambda t, k: (vstall, k * VROWS + t * 128),
                               vis_loads)
                vb = dp.tile([KD, 128, VCH // 4], F32, tag=f"vb{g}",
                             name=f"vb{g}")
                # scalar (qAct) HWDGE ring: empty, so the bounce isn't queued
                # behind input loads; its short wait stalls ACT only briefly
                bounces.append(nc.scalar.dma_start(
                    out=vb[:, :, :].rearrange("k p c -> p k c"),
                    in_=vst3[:, :, g * VCH:(g + 1) * VCH].bitcast(F32)))
                vg = dp.tile([N_CORES * KD, 128, VCH // 4], F32, tag=f"vg{g}",
                             name=f"vg{g}", addr_space="Shared")
                nc.gpsimd.collective_compute(
                    "AllGather", ALU.bypass, replica_groups=rg,
                    ins=[vb[:, :, :].opt()], outs=[vg[:, :, :].opt()])
                vgath.append(vg)

            # ---- audio prep ----------------------------------------------
            for t0 in range(0, NT_A, 8):
                prep_batch(a_in, t0, min(8, NT_A - t0),
                           lambda t, k: (aTf[k // 2],
                                         (k % 2) * AROWS + t * 128),
                           aud_loads1 if t0 == 0 else aud_loads2)

            # ---- load gathered visual into SBUF ---------------------------
            # vTall col (within d-chunk k) = g*(8*VCH) + i*VCH + (j*256 + m)
            #   -> holds global clip c = 4*i + g*CPC + j at cperm position
            for g in range(G):
                for i in range(N_CORES):
                    cola = g * (N_CORES * VCH) + i * VCH
                    for k in range(KD):
                        vt_loads.append(nc.sync.dma_start(
                            out=vTf[k // 2][:, (k % 2) * VTW + cola:
                                            (k % 2) * VTW + cola + VCH
                                            ].bitcast(F32),
                            in_=vgath[g][i * KD + k]))

            # Explicit sync-ring ordering: the HWDGE ring is FIFO per engine,
            # and a DMA whose wait isn't met blocks everything behind it.
            # Keep never-blocked input loads ahead of semaphore-gated loads.
            ring_groups = [
                vis_loads,
                aud_loads1,
                aud_loads2,
                vt_loads,
            ]
            prev = None
            for grp in ring_groups:
                if not grp:
                    continue
                if prev is not None:
                    for h in grp:
                        add_dep_helper(h.ins, prev.ins, sync=False,
                                       reason="sync-ring class order")
                prev = grp[-1]

            # ---- main loop: S = aT.T @ vT, rowmax, accumulate -------------
            # mx col layout: nt*128 + b*32 + cperm
            for g in range(G):
                for b in range(BL):
                    for nt in range(NA // 128):
                        lcol = (b * (NA // 128) + nt) * 128
                        for h in range(CPC):
                            base = g * (N_CORES * VCH) + h * 2048
                            pss = [ps.tile([128, 512], F32, tag="ps",
                                           name="mm") for _ in range(4)]
                            for k2 in range(KD2):
                                lhs3 = aTf[k2][:].rearrange(
                                    "p (ko m) -> p ko m", ko=2
                                )[:, :, lcol:lcol + 128]
                                for p in range(4):
                                    rhs3 = vTf[k2][:].rearrange(
                                        "p (ko n) -> p ko n", ko=2
                                    )[:, :, base + p * 512:
                                      base + (p + 1) * 512]
                                    nc.tensor.matmul(
                                        pss[p][:], lhsT=lhs3, rhs=rhs3,
                                        start=(k2 == 0), stop=(k2 == KD2 - 1),
                                        perf_mode=mybir.MatmulPerfMode.
                                        DoubleRow)
                            for p in range(4):
                                c0 = g * (8 * CPC) + (h * 4 + p) * 2
                                mcol = nt * 128 + b * 32 + c0
                                nc.vector.tensor_reduce(
                                    out=mx[:, mcol:mcol + 2],
                                    in_=pss[p][:].rearrange(
                                        "p (j m) -> p j m", j=2),
                                    axis=AX.X, op=ALU.max)

            # ---- column sums of row-maxes: mean over n --------------------
            pclip = ps.tile([1, mxw], F32, tag="ps", name="pclip")
            nc.tensor.matmul(pclip[:], lhsT=ones[:], rhs=mx[:],
                             start=True, stop=True)
            csum = wp.tile([1, 128], F32, tag="csum")
            nc.vector.tensor_reduce(
                out=csum[:],
                in_=pclip[:].rearrange("p (nt bc) -> p bc nt", nt=NA // 128),
                axis=AX.X, op=ALU.add)
            clip_blk = wp.tile([1, 128], F32, tag="clipblk")
            nc.scalar.mul(clip_blk[:], csum[:],
                          1.0 / (NA * TEMPERATURE * SCL * SCL))

            # ---- gather the (4,32) clip blocks ----------------------------
            cb = dp.tile([1, 128], F32, tag="cb", name="cb")
            nc.sync.dma_start(out=cb[:], in_=clip_blk[:])
            call = dp.tile([N_CORES, 128], F32, tag="call", name="call",
                           addr_space="Shared")
            nc.gpsimd.collective_compute(
                "AllGather", ALU.bypass, replica_groups=rg,
                ins=[cb[:, :].opt()], outs=[call[:, :].opt()])

            # ---- final loss (computed redundantly on every core) ----------
            clip_sb = wp.tile([32, 32], F32, tag="clip")
            nc.sync.dma_start(
                out=clip_sb[:],
                in_=call[:, :].rearrange("a (b c) -> (a b) c", b=4))
            pT = ps.tile([32, 32], F32, tag="ps", name="pT")
            nc.tensor.matmul(pT[:], lhsT=clip_sb[:], rhs=ident_f32[0:32, 0:32],
                             is_transpose=True)
            clipT = wp.tile([32, 32], F32, tag="clipT")
            nc.scalar.copy(clipT[:], pT[:])

            def lse_rows(x, nm_tag):
                # no max-stabilization: |clip| <= 1/T = 10, exp is safe in f32
                ex = wp.tile([32, 32], F32, tag=nm_tag + "ex", name="ex")
                es = wp.tile([32, 1], F32, tag=nm_tag + "es", name="es")
                nc.scalar.activation(ex[:], x[:], ACT.Exp, accum_out=es[:])
                lse = wp.tile([32, 1], F32, tag=nm_tag + "lse", name="lse")
                nc.scalar.activation(lse[:], es[:], ACT.Ln)
                return lse

            lse1 = lse_rows(clip_sb, "r")
            lse2 = lse_rows(clipT, "c")
            pmask_sb = wp.tile([32, 32], F32, tag="pmask")
            nc.sync.dma_start(out=pmask_sb[:], in_=pmask_dram[:])
            dsc = wp.tile([32, 32], F32, tag="dsc")
            diag = wp.tile([32, 1], F32, tag="diag")
            nc.vector.tensor_mul(dsc[:], clip_sb[:], pmask_sb[:])
            nc.vector.reduce_sum(out=diag[:], in_=dsc[:], axis=AX.X)
            s = wp.tile([32, 1], F32, tag="s")
            nc.vector.tensor_add(s[:], lse1[:], lse2[:])
            lb = wp.tile([32, 1], F32, tag="lb")
            nc.vector.scalar_tensor_tensor(
                out=lb[:], in0=s[:], scalar=0.5, in1=diag[:],
                op0=ALU.mult, op1=ALU.subtract)
            pl = ps.tile([1, 1], F32, tag="ps", name="pl")
            nc.tensor.matmul(pl[:], lhsT=ones[0:32, :], rhs=lb[:],
                             start=True, stop=True)
            res = wp.tile([1, 1], F32, tag="res")
            nc.scalar.mul(res[:], pl[:], 1.0 / B)
            nc.sync.dma_start(out=out[:], in_=res[:])

    nc.finalize()
    return nc


_NC_CACHE = None


def kernel(audio_feats: np.ndarray, visual_feats: np.ndarray) -> np.ndarray:
    from concourse.bass_utils import run_bass_kernel_spmd

    global _NC_CACHE
    if _NC_CACHE is None:
        _NC_CACHE = build()
    nc = _NC_CACHE

    audio = np.ascontiguousarray(audio_feats, dtype=np.float32)
    visual = np.ascontiguousarray(visual_feats, dtype=np.float32)
    in_maps = []
    for i in range(N_CORES):
        in_maps.append({
            "audio": audio[i * BL:(i + 1) * BL].reshape(AROWS, D),
            "visual": visual[i * BL:(i + 1) * BL].reshape(VROWS, D),
        })
    res = run_bass_kernel_spmd(nc, in_maps, core_ids=list(range(N_CORES)))
    val = res.results[0]["out"][0, 0]
    return np.asarray(val, dtype=np.float32)


if __name__ == "__main__":
    rng = np.random.default_rng(0)
    a = rng.standard_normal((B, NA, D)).astype(np.float32)
    v = rng.standard_normal((B, NV, D)).astype(np.float32)
    print(kernel(a, v))


# revision 50
# speedup vs baseline: 1.6255x; 1.1315x over previous
"""Distributed Trainium2 kernel for the audio-visual contrastive loss.

Math (reference):
    a = l2norm(audio)  (B=32, Na=512, D=768)
    v = l2norm(visual) (B=32, Nv=256, D=768)
    token_sims[b,c,n,m] = (a[b,n] . v[c,m]) / T
    clip_sims = mean_n max_m token_sims          (B, B)
    loss = mean_b -0.5*(log_softmax(clip)[b,b] + log_softmax(clip.T)[b,b])

Distribution over 8 NeuronCores:
    - audio batch is sharded 4 clips/core; visual batch is sharded 4 clips/core
      for the (normalize + transpose) prep, then AllGather'd (as bf16, d-major)
      in 2 chunks so the second chunk's collective overlaps the first chunk's
      matmuls.
    - each core computes its (4, 32) block of clip_sims:
        S[n, m] = aT[:, n].T @ vT[:, m]  accumulated over 6 d-chunks in PSUM,
        row-max over m on VectorE, column-sum over n via a ones-matmul.
    - the (4,32) blocks are AllGather'd (tiny) and every core computes the
      final scalar loss redundantly.
"""

import os
import sys

for _p in ("/opt/trn_rl_repo",):
    if _p not in sys.path:
        sys.path.insert(0, _p)

import numpy as np

import concourse.bacc as bacc
import concourse.mybir as mybir
import concourse.tile as tile
from concourse.tile_rust import add_dep_helper

N_CORES = 8
B = 32
NA = int(os.environ.get("KERNEL_NA", "512"))
NV = 256
D = int(os.environ.get("KERNEL_D", "768"))
TEMPERATURE = 0.1
BL = B // N_CORES            # 4 clips per core
AROWS = BL * NA              # 2048 audio rows per core
VROWS = BL * NV              # 1024 visual rows per core
KD = D // 128                # 6 contraction chunks
NT_A = AROWS // 128          # 16 audio row-tiles
NT_V = VROWS // 128          # 8 visual row-tiles
G = int(os.environ.get("KERNEL_GATHER_CHUNKS", "2"))  # visual AllGather chunks
VCH = VROWS // G             # visual rows per chunk per core
CPC = BL // G                # clips per chunk per core

F32 = mybir.dt.float32
BF16 = mybir.dt.bfloat16
FP8 = mybir.dt.float8e4
AX = mybir.AxisListType
ALU = mybir.AluOpType
ACT = mybir.ActivationFunctionType
SCL = 16.0                   # fp8 pre-scale (folded into the norm rsqrt)


def build():
    nc = bacc.Bacc("TRN2", target_bir_lowering=False, debug=False,
                   num_devices=N_CORES)
    a_in = nc.declare_dram_parameter("audio", [AROWS, D], F32, isOutput=False)
    v_in = nc.declare_dram_parameter("visual", [VROWS, D], F32, isOutput=False)
    out = nc.declare_dram_parameter("out", [1, 1], F32, isOutput=True)
    ident_dram = nc.inline_tensor(np.eye(128, dtype=np.float32), name="ident")
    # vT/mx columns hold clips in "production order"
    #   cperm(c) = g*(8*CPC) + i*CPC + j  for global clip c = 4*i + g*CPC + j.
    # Row/column logsumexp sums are permutation-invariant; only the diagonal
    # extraction needs the map, via this permuted identity.
    pmask = np.zeros((32, 32), dtype=np.float32)
    for c in range(B):
        i, q = divmod(c, 4)
        g, j = divmod(q, CPC)
        pmask[c, g * (8 * CPC) + i * CPC + j] = 1.0
    pmask_dram = nc.inline_tensor(pmask, name="pmask")
    rg = [list(range(N_CORES))]

    with tile.TileContext(nc) as tc:
        with (
            tc.tile_pool(name="persist", bufs=1) as pp,
            tc.tile_pool(name="work", bufs=3) as wp,
            tc.tile_pool(name="ps", bufs=8, space="PSUM") as ps,
            tc.tile_pool(name="dram", bufs=1, space="DRAM") as dp,
        ):
            # ---- constants -------------------------------------------------
            ident_f32 = pp.tile([128, 128], F32, tag="identf")
            nc.sync.dma_start(out=ident_f32[:], in_=ident_dram[:])
            ident_bf = pp.tile([128, 128], BF16, tag="identb")
            nc.scalar.copy(ident_bf[:], ident_f32[:])
            ones = pp.tile([128, 1], F32, tag="ones")
            nc.gpsimd.memset(ones[:], 1.0)

            # ---- persistent tensors ---------------------------------------
            # fp8 operands in DoubleRow ko-paired layout: tile k2 holds
            # d-chunk 2*k2 at ko=0 and 2*k2+1 at ko=1 (column offset AROWS/VTW)
            VTW = N_CORES * VROWS        # 8192 vT columns per d-chunk
            KD2 = KD // 2
            aTf = [pp.tile([128, 2 * AROWS], FP8, tag=f"aT8{k2}",
                           name=f"aT8{k2}") for k2 in range(KD2)]
            # single tile so bounce DMAs can stride across d-chunks
            vstall = pp.tile([128, KD * VROWS], FP8, tag="vstall")
            # separate per-(k2) gather destinations keep subtile dependency
            # tracking fine-grained for the matmul reads
            vTf = [pp.tile([128, 2 * VTW], FP8, tag=f"vT8{k2}",
                           name=f"vT8{k2}") for k2 in range(KD2)]
            mxw = (NA // 128) * 128
            mx = pp.tile([128, mxw], F32, tag="mx")

            # warmup collective: absorbs first-collective staging latency
            # while the input DMAs run
            wu_in = dp.tile([1, 32], F32, tag="wu_in", name="wu_in")
            wu_out = dp.tile([N_CORES, 32], F32, tag="wu_out", name="wu_out",
                             addr_space="Shared")
            wu_sb = pp.tile([1, 32], F32, tag="wu_sb")
            nc.gpsimd.memset(wu_sb[:], 0.0)
            nc.gpsimd.dma_start(out=wu_in[:], in_=wu_sb[:])
            nc.gpsimd.collective_compute(
                "AllGather", ALU.bypass, replica_groups=rg,
                ins=[wu_in[:, :].opt()], outs=[wu_out[:, :].opt()])

            # ---- row-tile prep: normalize rows, cast bf16, transpose ------
            # Row-tile prep, batched per-op so each engine runs bursts of the
            # same instruction instead of per-tile cross-engine round trips:
            #   wave of 8: DMA loads -> ACT Square(accum=ss) -> ACT sqrt ->
            #   DVE reciprocal -> ACT scaled casts -> PE transposes ->
            #   DVE psum->sbuf copies
            def prep_batch(src, t0, nb, dst_of, load_group):
                raws = []
                ssb = wp.tile([128, nb], F32, tag="ssb", name="ssb", bufs=2)
                for j in range(nb):
                    t = t0 + j
                    raw = wp.tile([128, D], F32, tag="raw", name="raw", bufs=8)
                    load_group.append(
                        nc.sync.dma_start(out=raw[:],
                                          in_=src[t * 128:(t + 1) * 128, :]))
                    sqs = wp.tile([128, D], F32, tag="sqs", name="sqs", bufs=2)
                    nc.scalar.activation(sqs[:], raw[:], ACT.Square,
                                         accum_out=ssb[:, j:j + 1])
                    raws.append(raw)
                nrm = wp.tile([128, nb], F32, tag="nrm", name="nrm", bufs=2)
                # norm/SCL, so 1/nrm scales rows by SCL/||x|| (fp8 pre-scale)
                nc.scalar.activation(nrm[:], ssb[:], ACT.Sqrt,
                                     scale=1.0 / (SCL * SCL))
                rnb = wp.tile([128, nb], F32, tag="rnb", name="rnb", bufs=2)
                nc.vector.reciprocal(rnb[:], nrm[:])
                for j in range(nb):
                    t = t0 + j
                    nbf = wp.tile([128, D], BF16, tag="nbf", name="nbf",
                                  bufs=4)
                    nc.scalar.activation(nbf[:], raws[j][:], ACT.Copy,
                                         bias=0.0, scale=rnb[:, j:j + 1])
                    for k in range(KD):
                        pt = ps.tile([128, 128], BF16, tag="ps", name="pt")
                        nc.tensor.transpose(pt[:],
                                            nbf[:, 128 * k:128 * (k + 1)],
                                            ident_bf[:])
                        dst_tile, col = dst_of(t, k)
                        nc.vector.tensor_copy(dst_tile[:, col:col + 128],
                                              pt[:])

            # ---- visual prep + bounce + chunked AllGather -----------------
            # DMA ring discipline (head-of-line blocking avoidance):
            #   nc.sync   : input loads only (never blocked by a semaphore)
            #   nc.gpsimd : bounce writes + collectives (SWDGE)
            #   nc.scalar : gathered-visual loads (qActDynamicHW; their AG
            #               waits land after all prep compute on ACT)
            # Gather buffers are f32-typed views (bitcast) of the bf16 data.
            nbv = max(2, NT_V // G)      # visual prep batch = one chunk
            vis_loads, aud_loads1, aud_loads2 = [], [], []
            bounces, vt_loads = [], []
            vgath = []
            vst3 = vstall[:].rearrange("p (k c) -> p k c", k=KD)
            for g in range(G):
                for t0 in range(g * (NT_V // G), (g + 1) * (NT_V // G), nbv):
                    prep_batch(v_in, t0, nbv,
                               lambda t, k: (vstall, k * VROWS + t * 128),
                               vis_loads)
                vb = dp.tile([KD, 128, VCH // 4], F32, tag=f"vb{g}",
                             name=f"vb{g}")
                # scalar (qAct) HWDGE ring: empty, so the bounce isn't queued
                # behind input loads; its short wait stalls ACT only briefly
                bounces.append(nc.scalar.dma_start(
                    out=vb[:, :, :].rearrange("k p c -> p k c"),
                    in_=vst3[:, :, g * VCH:(g + 1) * VCH].bitcast(F32)))
                vg = dp.tile([N_CORES * KD, 128, VCH // 4], F32, tag=f"vg{g}",
                             name=f"vg{g}", addr_space="Shared")
                nc.gpsimd.collective_compute(
                    "AllGather", ALU.bypass, replica_groups=rg,
                    ins=[vb[:, :, :].opt()], outs=[vg[:, :, :].opt()])
                vgath.append(vg)

            # ---- audio prep ----------------------------------------------
            for t0 in range(0, NT_A, 8):
                prep_batch(a_in, t0, min(8, NT_A - t0),
                           lambda t, k: (aTf[k // 2],
                                         (k % 2) * AROWS + t * 128),
                           aud_loads1 if t0 == 0 else aud_loads2)

            # ---- load gathered visual into SBUF ---------------------------
            # vTall col (within d-chunk k) = g*(8*VCH) + i*VCH + (j*256 + m)
            #   -> holds global clip c = 4*i + g*CPC + j at cperm position
            for g in range(G):
                for i in range(N_CORES):
                    cola = g * (N_CORES * VCH) + i * VCH
                    for k in range(KD):
                        vt_loads.append(nc.sync.dma_start(
                            out=vTf[k // 2][:, (k % 2) * VTW + cola:
                                            (k % 2) * VTW + cola + VCH
                                            ].bitcast(F32),
                            in_=vgath[g][i * KD + k]))

            # Explicit sync-ring ordering: the HWDGE ring is FIFO per engine,
            # and a DMA whose wait isn't met blocks everything behind it.
            # Keep never-blocked input loads ahead of semaphore-gated loads.
            ring_groups = [
                vis_loads,
                aud_loads1,
                aud_loads2,
                vt_loads,
            ]
            prev = None
            for grp in ring_groups:
                if not grp:
                    continue
                if prev is not None:
                    for h in grp:
                        add_dep_helper(h.ins, prev.ins, sync=False,
                                       reason="sync-ring class order")
                prev = grp[-1]

            # ---- main loop: S = aT.T @ vT, rowmax, accumulate -------------
            # mx col layout: nt*128 + b*32 + cperm
            for g in range(G):
                for b in range(BL):
                    for nt in range(NA // 128):
                        lcol = (b * (NA // 128) + nt) * 128
                        for h in range(CPC):
                            base = g * (N_CORES * VCH) + h * 2048
                            pss = [ps.tile([128, 512], F32, tag="ps",
                                           name="mm") for _ in range(4)]
                            for k2 in range(KD2):
                                lhs3 = aTf[k2][:].rearrange(
                                    "p (ko m) -> p ko m", ko=2
                                )[:, :, lcol:lcol + 128]
                                for p in range(4):
                                    rhs3 = vTf[k2][:].rearrange(
                                        "p (ko n) -> p ko n", ko=2
                                    )[:, :, base + p * 512:
                                      base + (p + 1) * 512]
                                    nc.tensor.matmul(
                                        pss[p][:], lhsT=lhs3, rhs=rhs3,
                                        start=(k2 == 0), stop=(k2 == KD2 - 1),
                                        perf_mode=mybir.MatmulPerfMode.
                                        DoubleRow)
                            for p in range(4):
                                c0 = g * (8 * CPC) + (h * 4 + p) * 2
                                mcol = nt * 128 + b * 32 + c0
                                nc.vector.tensor_reduce(
                                    out=mx[:, mcol:mcol + 2],
                                    in_=pss[p][:].rearrange(
                                        "p (j m) -> p j m", j=2),
                                    axis=AX.X, op=ALU.max)

            # ---- column sums of row-maxes: mean over n --------------------
            pclip = ps.tile([1, mxw], F32, tag="ps", name="pclip")
            nc.tensor.matmul(pclip[:], lhsT=ones[:], rhs=mx[:],
                             start=True, stop=True)
            csum = wp.tile([1, 128], F32, tag="csum")
            nc.vector.tensor_reduce(
                out=csum[:],
                in_=pclip[:].rearrange("p (nt bc) -> p bc nt", nt=NA // 128),
                axis=AX.X, op=ALU.add)
            clip_blk = wp.tile([1, 128], F32, tag="clipblk")
            nc.scalar.mul(clip_blk[:], csum[:],
                          1.0 / (NA * TEMPERATURE * SCL * SCL))

            # ---- gather the (4,32) clip blocks ----------------------------
            cb = dp.tile([1, 128], F32, tag="cb", name="cb")
            nc.sync.dma_start(out=cb[:], in_=clip_blk[:])
            call = dp.tile([N_CORES, 128], F32, tag="call", name="call",
                           addr_space="Shared")
            nc.gpsimd.collective_compute(
                "AllGather", ALU.bypass, replica_groups=rg,
                ins=[cb[:, :].opt()], outs=[call[:, :].opt()])

            # ---- final loss (computed redundantly on every core) ----------
            clip_sb = wp.tile([32, 32], F32, tag="clip")
            nc.sync.dma_start(
                out=clip_sb[:],
                in_=call[:, :].rearrange("a (b c) -> (a b) c", b=4))
            pT = ps.tile([32, 32], F32, tag="ps", name="pT")
            nc.tensor.matmul(pT[:], lhsT=clip_sb[:], rhs=ident_f32[0:32, 0:32],
                             is_transpose=True)
            clipT = wp.tile([32, 32], F32, tag="clipT")
            nc.scalar.copy(clipT[:], pT[:])

            def lse_rows(x, nm_tag):
                # no max-stabilization: |clip| <= 1/T = 10, exp is safe in f32
                ex = wp.tile([32, 32], F32, tag=nm_tag + "ex", name="ex")
                es = wp.tile([32, 1], F32, tag=nm_tag + "es", name="es")
                nc.scalar.activation(ex[:], x[:], ACT.Exp, accum_out=es[:])
                lse = wp.tile([32, 1], F32, tag=nm_tag + "lse", name="lse")
                nc.scalar.activation(lse[:], es[:], ACT.Ln)
                return lse

            lse1 = lse_rows(clip_sb, "r")
            lse2 = lse_rows(clipT, "c")
            pmask_sb = wp.tile([32, 32], F32, tag="pmask")
            nc.sync.dma_start(out=pmask_sb[:], in_=pmask_dram[:])
            dsc = wp.tile([32, 32], F32, tag="dsc")
            diag = wp.tile([32, 1], F32, tag="diag")
            nc.vector.tensor_mul(dsc[:], clip_sb[:], pmask_sb[:])
            nc.vector.reduce_sum(out=diag[:], in_=dsc[:], axis=AX.X)
            s = wp.tile([32, 1], F32, tag="s")
            nc.vector.tensor_add(s[:], lse1[:], lse2[:])
            lb = wp.tile([32, 1], F32, tag="lb")
            nc.vector.scalar_tensor_tensor(
                out=lb[:], in0=s[:], scalar=0.5, in1=diag[:],
                op0=ALU.mult, op1=ALU.subtract)
            pl = ps.tile([1, 1], F32, tag="ps", name="pl")
            nc.tensor.matmul(pl[:], lhsT=ones[0:32, :], rhs=lb[:],
                             start=True, stop=True)
            res = wp.tile([1, 1], F32, tag="res")
            nc.scalar.mul(res[:], pl[:], 1.0 / B)
            nc.sync.dma_start(out=out[:], in_=res[:])

    nc.finalize()
    return nc


_NC_CACHE = None


def kernel(audio_feats: np.ndarray, visual_feats: np.ndarray) -> np.ndarray:
    from concourse.bass_utils import run_bass_kernel_spmd

    global _NC_CACHE
    if _NC_CACHE is None:
        _NC_CACHE = build()
    nc = _NC_CACHE

    audio = np.ascontiguousarray(audio_feats, dtype=np.float32)
    visual = np.ascontiguousarray(visual_feats, dtype=np.float32)
    in_maps = []
    for i in range(N_CORES):
        in_maps.append({
            "audio": audio[i * BL:(i + 1) * BL].reshape(AROWS, D),
            "visual": visual[i * BL:(i + 1) * BL].reshape(VROWS, D),
        })
    res = run_bass_kernel_spmd(nc, in_maps, core_ids=list(range(N_CORES)))
    val = res.results[0]["out"][0, 0]
    return np.asarray(val, dtype=np.float32)


if __name__ == "__main__":
    rng = np.random.default_rng(0)
    a = rng.standard_normal((B, NA, D)).astype(np.float32)
    v = rng.standard_normal((B, NV, D)).astype(np.float32)
    print(kernel(a, v))


# revision 55
# speedup vs baseline: 1.8683x; 1.1494x over previous
"""Distributed Trainium2 kernel for the audio-visual contrastive loss.

Math (reference):
    a = l2norm(audio)  (B=32, Na=512, D=768)
    v = l2norm(visual) (B=32, Nv=256, D=768)
    token_sims[b,c,n,m] = (a[b,n] . v[c,m]) / T
    clip_sims = mean_n max_m token_sims          (B, B)
    loss = mean_b -0.5*(log_softmax(clip)[b,b] + log_softmax(clip.T)[b,b])

Distribution over 8 NeuronCores:
    - audio batch is sharded 4 clips/core; visual batch is sharded 4 clips/core
      for the (normalize + transpose) prep, then AllGather'd (as bf16, d-major)
      in 2 chunks so the second chunk's collective overlaps the first chunk's
      matmuls.
    - each core computes its (4, 32) block of clip_sims:
        S[n, m] = aT[:, n].T @ vT[:, m]  accumulated over 6 d-chunks in PSUM,
        row-max over m on VectorE, column-sum over n via a ones-matmul.
    - the (4,32) blocks are AllGather'd (tiny) and every core computes the
      final scalar loss redundantly.
"""

import os
import sys

for _p in ("/opt/trn_rl_repo",):
    if _p not in sys.path:
        sys.path.insert(0, _p)

import numpy as np

import concourse.bacc as bacc
import concourse.mybir as mybir
import concourse.tile as tile
from concourse.tile_rust import add_dep_helper

N_CORES = 8
B = 32
NA = int(os.environ.get("KERNEL_NA", "512"))
NV = 256
D = int(os.environ.get("KERNEL_D", "768"))
TEMPERATURE = 0.1
BL = B // N_CORES            # 4 clips per core
AROWS = BL * NA              # 2048 audio rows per core
VROWS = BL * NV              # 1024 visual rows per core
KD = D // 128                # 6 contraction chunks
NT_A = AROWS // 128          # 16 audio row-tiles
NT_V = VROWS // 128          # 8 visual row-tiles
G = int(os.environ.get("KERNEL_GATHER_CHUNKS", "2"))  # visual AllGather chunks
VCH = VROWS // G             # visual rows per chunk per core
CPC = BL // G                # clips per chunk per core

F32 = mybir.dt.float32
BF16 = mybir.dt.bfloat16
FP8 = mybir.dt.float8e4
AX = mybir.AxisListType
ALU = mybir.AluOpType
ACT = mybir.ActivationFunctionType
SCL = 16.0                   # fp8 pre-scale (folded into the norm rsqrt)


def build():
    nc = bacc.Bacc("TRN2", target_bir_lowering=False, debug=False,
                   num_devices=N_CORES)
    a_in = nc.declare_dram_parameter("audio", [AROWS, D], F32, isOutput=False)
    v_in = nc.declare_dram_parameter("visual", [VROWS, D], F32, isOutput=False)
    out = nc.declare_dram_parameter("out", [1, 1], F32, isOutput=True)
    ident_dram = nc.inline_tensor(np.eye(128, dtype=np.float32), name="ident")
    # vT/mx columns hold clips in "production order"
    #   cperm(c) = g*(8*CPC) + i*CPC + j  for global clip c = 4*i + g*CPC + j.
    # Row/column logsumexp sums are permutation-invariant; only the diagonal
    # extraction needs the map, via this permuted identity.
    pmask = np.zeros((32, 32), dtype=np.float32)
    for c in range(B):
        i, q = divmod(c, 4)
        g, j = divmod(q, CPC)
        pmask[c, g * (8 * CPC) + i * CPC + j] = 1.0
    pmask_dram = nc.inline_tensor(pmask, name="pmask")
    rg = [list(range(N_CORES))]

    with tile.TileContext(nc) as tc:
        with (
            tc.tile_pool(name="persist", bufs=1) as pp,
            tc.tile_pool(name="work", bufs=3) as wp,
            tc.tile_pool(name="ps", bufs=8, space="PSUM") as ps,
            tc.tile_pool(name="dram", bufs=1, space="DRAM") as dp,
        ):
            # ---- constants -------------------------------------------------
            ident_f32 = pp.tile([128, 128], F32, tag="identf")
            nc.sync.dma_start(out=ident_f32[:], in_=ident_dram[:])
            ident_bf = pp.tile([128, 128], BF16, tag="identb")
            nc.scalar.copy(ident_bf[:], ident_f32[:])
            ones = pp.tile([128, 1], F32, tag="ones")
            nc.gpsimd.memset(ones[:], 1.0)

            # ---- persistent tensors ---------------------------------------
            # fp8 operands in DoubleRow ko-paired layout: tile k2 holds
            # d-chunk 2*k2 at ko=0 and 2*k2+1 at ko=1 (column offset AROWS/VTW)
            VTW = N_CORES * VROWS        # 8192 vT columns per d-chunk
            KD2 = KD // 2
            aTf = [pp.tile([128, 2 * AROWS], FP8, tag=f"aT8{k2}",
                           name=f"aT8{k2}") for k2 in range(KD2)]
            # single tile so bounce DMAs can stride across d-chunks
            vstall = pp.tile([128, KD * VROWS], FP8, tag="vstall")
            # separate per-(k2) gather destinations keep subtile dependency
            # tracking fine-grained for the matmul reads
            vTf = [pp.tile([128, 2 * VTW], FP8, tag=f"vT8{k2}",
                           name=f"vT8{k2}") for k2 in range(KD2)]
            mxw = (NA // 128) * 128
            mx = pp.tile([128, mxw], F32, tag="mx")

            # warmup collective: absorbs first-collective staging latency
            # while the input DMAs run
            wu_in = dp.tile([1, 32], F32, tag="wu_in", name="wu_in")
            wu_out = dp.tile([N_CORES, 32], F32, tag="wu_out", name="wu_out",
                             addr_space="Shared")
            wu_sb = pp.tile([1, 32], F32, tag="wu_sb")
            nc.gpsimd.memset(wu_sb[:], 0.0)
            nc.gpsimd.dma_start(out=wu_in[:], in_=wu_sb[:])
            nc.gpsimd.collective_compute(
                "AllGather", ALU.bypass, replica_groups=rg,
                ins=[wu_in[:, :].opt()], outs=[wu_out[:, :].opt()])

            # ---- row-tile prep: normalize rows, cast bf16, transpose ------
            # Row-tile prep, batched per-op so each engine runs bursts of the
            # same instruction instead of per-tile cross-engine round trips:
            #   wave of 8: DMA loads -> ACT Square(accum=ss) -> ACT sqrt ->
            #   DVE reciprocal -> ACT scaled casts -> PE transposes ->
            #   DVE psum->sbuf copies
            def prep_batch(src, t0, nb, dst_of, load_group):
                raws = []
                ssb = wp.tile([128, nb], F32, tag="ssb", name="ssb", bufs=2)
                for j in range(nb):
                    t = t0 + j
                    raw = wp.tile([128, D], F32, tag="raw", name="raw", bufs=8)
                    load_group.append(
                        nc.sync.dma_start(out=raw[:],
                                          in_=src[t * 128:(t + 1) * 128, :]))
                    sqs = wp.tile([128, D], F32, tag="sqs", name="sqs", bufs=2)
                    nc.scalar.activation(sqs[:], raw[:], ACT.Square,
                                         accum_out=ssb[:, j:j + 1])
                    raws.append(raw)
                nrm = wp.tile([128, nb], F32, tag="nrm", name="nrm", bufs=2)
                # norm/SCL, so 1/nrm scales rows by SCL/||x|| (fp8 pre-scale)
                nc.scalar.activation(nrm[:], ssb[:], ACT.Sqrt,
                                     scale=1.0 / (SCL * SCL))
                rnb = wp.tile([128, nb], F32, tag="rnb", name="rnb", bufs=2)
                nc.vector.reciprocal(rnb[:], nrm[:])
                for j in range(nb):
                    t = t0 + j
                    nbf = wp.tile([128, D], BF16, tag="nbf", name="nbf",
                                  bufs=4)
                    nc.scalar.activation(nbf[:], raws[j][:], ACT.Copy,
                                         bias=0.0, scale=rnb[:, j:j + 1])
                    for k in range(KD):
                        pt = ps.tile([128, 128], BF16, tag="ps", name="pt")
                        nc.tensor.transpose(pt[:],
                                            nbf[:, 128 * k:128 * (k + 1)],
                                            ident_bf[:])
                        dst_tile, col = dst_of(t, k)
                        nc.vector.tensor_copy(dst_tile[:, col:col + 128],
                                              pt[:])

            # ---- visual prep + bounce + chunked AllGather -----------------
            # DMA ring discipline (head-of-line blocking avoidance):
            #   nc.sync   : input loads only (never blocked by a semaphore)
            #   nc.gpsimd : bounce writes + collectives (SWDGE)
            #   nc.scalar : gathered-visual loads (qActDynamicHW; their AG
            #               waits land after all prep compute on ACT)
            # Gather buffers are f32-typed views (bitcast) of the bf16 data.
            nbv = max(2, NT_V // G)      # visual prep batch = one chunk
            vis_loads, aud_loads1, aud_loads2 = [], [], []
            bounces, vt_loads = [], []
            vgath = []
            vst3 = vstall[:].rearrange("p (k c) -> p k c", k=KD)
            for g in range(G):
                for t0 in range(g * (NT_V // G), (g + 1) * (NT_V // G), nbv):
                    prep_batch(v_in, t0, nbv,
                               lambda t, k: (vstall, k * VROWS + t * 128),
                               vis_loads)
                # partition-major bounce rows: row p holds its full (k, c)
                # strip contiguously, so the post-gather loads get 1KB
                # contiguous runs on both sides
                vb = dp.tile([128, KD * VCH // 4], F32, tag=f"vb{g}",
                             name=f"vb{g}")
                # scalar (qAct) HWDGE ring: empty, so the bounce isn't queued
                # behind input loads; its short wait stalls ACT only briefly
                bounces.append(nc.scalar.dma_start(
                    out=vb[:, :].rearrange("p (k c) -> p k c", k=KD),
                    in_=vst3[:, :, g * VCH:(g + 1) * VCH].bitcast(F32)))
                vg = dp.tile([N_CORES * 128, KD * VCH // 4], F32,
                             tag=f"vg{g}", name=f"vg{g}", addr_space="Shared")
                nc.gpsimd.collective_compute(
                    "AllGather", ALU.bypass, replica_groups=rg,
                    ins=[vb[:, :].opt()], outs=[vg[:, :].opt()])
                vgath.append(vg)

            # ---- audio prep ----------------------------------------------
            for t0 in range(0, NT_A, 8):
                prep_batch(a_in, t0, min(8, NT_A - t0),
                           lambda t, k: (aTf[k // 2],
                                         (k % 2) * AROWS + t * 128),
                           aud_loads1 if t0 == 0 else aud_loads2)

            # ---- load gathered visual into SBUF ---------------------------
            # vTall col (within d-chunk k) = g*(8*VCH) + i*VCH + (j*256 + m)
            #   -> holds global clip c = 4*i + g*CPC + j at cperm position
            # vTf[k2] free layout: 512-col n-blocks with the two ko halves
            # adjacent: col = f*1024 + ko*512 + n, where block f holds the
            # clip pair (2f, 2f+1) in cperm order. Both sides of each load
            # are 1KB-contiguous per partition.
            for g in range(G):
                for i in range(N_CORES):
                    nblk = VCH // 512     # 512-col n-blocks per (g, i)
                    blk3 = vgath[g][i * 128:(i + 1) * 128, :].rearrange(
                        "p (k c) -> p k c", k=KD)
                    for jb in range(nblk):
                        f = (g * N_CORES + i) * nblk + jb
                        for k2 in range(KD2):
                            vt_loads.append(nc.sync.dma_start(
                                out=vTf[k2][:, f * 1024:
                                            (f + 1) * 1024].bitcast(F32),
                                in_=blk3[:, 2 * k2:2 * k2 + 2,
                                         jb * 128:(jb + 1) * 128]))

            # Explicit sync-ring ordering: the HWDGE ring is FIFO per engine,
            # and a DMA whose wait isn't met blocks everything behind it.
            # Keep never-blocked input loads ahead of semaphore-gated loads.
            ring_groups = [
                vis_loads,
                aud_loads1,
                aud_loads2,
                vt_loads,
            ]
            prev = None
            for grp in ring_groups:
                if not grp:
                    continue
                if prev is not None:
                    for h in grp:
                        add_dep_helper(h.ins, prev.ins, sync=False,
                                       reason="sync-ring class order")
                prev = grp[-1]

            # ---- main loop: S = aT.T @ vT, rowmax, accumulate -------------
            # mx col layout: nt*128 + b*32 + cperm
            for g in range(G):
                for b in range(BL):
                    for nt in range(NA // 128):
                        lcol = (b * (NA // 128) + nt) * 128
                        for h in range(CPC):
                            pss = [ps.tile([128, 512], F32, tag="ps",
                                           name="mm") for _ in range(4)]
                            for k2 in range(KD2):
                                lhs3 = aTf[k2][:].rearrange(
                                    "p (ko m) -> p ko m", ko=2
                                )[:, :, lcol:lcol + 128]
                                for p in range(4):
                                    f = g * 4 * CPC + h * 4 + p
                                    rhs3 = vTf[k2][:].rearrange(
                                        "p (f ko n) -> p f ko n",
                                        ko=2, n=512)[:, f]
                                    nc.tensor.matmul(
                                        pss[p][:], lhsT=lhs3, rhs=rhs3,
                                        start=(k2 == 0), stop=(k2 == KD2 - 1),
                                        perf_mode=mybir.MatmulPerfMode.
                                        DoubleRow)
                            for p in range(4):
                                c0 = g * (8 * CPC) + (h * 4 + p) * 2
                                mcol = nt * 128 + b * 32 + c0
                                nc.vector.tensor_reduce(
                                    out=mx[:, mcol:mcol + 2],
                                    in_=pss[p][:].rearrange(
                                        "p (j m) -> p j m", j=2),
                                    axis=AX.X, op=ALU.max)

            # ---- column sums of row-maxes: mean over n --------------------
            pclip = ps.tile([1, mxw], F32, tag="ps", name="pclip")
            nc.tensor.matmul(pclip[:], lhsT=ones[:], rhs=mx[:],
                             start=True, stop=True)
            csum = wp.tile([1, 128], F32, tag="csum")
            nc.vector.tensor_reduce(
                out=csum[:],
                in_=pclip[:].rearrange("p (nt bc) -> p bc nt", nt=NA // 128),
                axis=AX.X, op=ALU.add)
            clip_blk = wp.tile([1, 128], F32, tag="clipblk")
            nc.scalar.mul(clip_blk[:], csum[:],
                          1.0 / (NA * TEMPERATURE * SCL * SCL))

            # ---- gather the (4,32) clip blocks ----------------------------
            cb = dp.tile([1, 128], F32, tag="cb", name="cb")
            nc.sync.dma_start(out=cb[:], in_=clip_blk[:])
            call = dp.tile([N_CORES, 128], F32, tag="call", name="call",
                           addr_space="Shared")
            nc.gpsimd.collective_compute(
                "AllGather", ALU.bypass, replica_groups=rg,
                ins=[cb[:, :].opt()], outs=[call[:, :].opt()])

            # ---- final loss (computed redundantly on every core) ----------
            clip_sb = wp.tile([32, 32], F32, tag="clip")
            nc.sync.dma_start(
                out=clip_sb[:],
                in_=call[:, :].rearrange("a (b c) -> (a b) c", b=4))
            pT = ps.tile([32, 32], F32, tag="ps", name="pT")
            nc.tensor.matmul(pT[:], lhsT=clip_sb[:], rhs=ident_f32[0:32, 0:32],
                             is_transpose=True)
            clipT = wp.tile([32, 32], F32, tag="clipT")
            nc.scalar.copy(clipT[:], pT[:])

            def lse_rows(x, nm_tag):
                # no max-stabilization: |clip| <= 1/T = 10, exp is safe in f32
                ex = wp.tile([32, 32], F32, tag=nm_tag + "ex", name="ex")
                es = wp.tile([32, 1], F32, tag=nm_tag + "es", name="es")
                nc.scalar.activation(ex[:], x[:], ACT.Exp, accum_out=es[:])
                lse = wp.tile([32, 1], F32, tag=nm_tag + "lse", name="lse")
                nc.scalar.activation(lse[:], es[:], ACT.Ln)
                return lse

            lse1 = lse_rows(clip_sb, "r")
            lse2 = lse_rows(clipT, "c")
            pmask_sb = wp.tile([32, 32], F32, tag="pmask")
            nc.sync.dma_start(out=pmask_sb[:], in_=pmask_dram[:])
            dsc = wp.tile([32, 32], F32, tag="dsc")
            diag = wp.tile([32, 1], F32, tag="diag")
            nc.vector.tensor_mul(dsc[:], clip_sb[:], pmask_sb[:])
            nc.vector.reduce_sum(out=diag[:], in_=dsc[:], axis=AX.X)
            s = wp.tile([32, 1], F32, tag="s")
            nc.vector.tensor_add(s[:], lse1[:], lse2[:])
            lb = wp.tile([32, 1], F32, tag="lb")
            nc.vector.scalar_tensor_tensor(
                out=lb[:], in0=s[:], scalar=0.5, in1=diag[:],
                op0=ALU.mult, op1=ALU.subtract)
            pl = ps.tile([1, 1], F32, tag="ps", name="pl")
            nc.tensor.matmul(pl[:], lhsT=ones[0:32, :], rhs=lb[:],
                             start=True, stop=True)
            res = wp.tile([1, 1], F32, tag="res")
            nc.scalar.mul(res[:], pl[:], 1.0 / B)
            nc.sync.dma_start(out=out[:], in_=res[:])

    nc.finalize()
    return nc


_NC_CACHE = None


def kernel(audio_feats: np.ndarray, visual_feats: np.ndarray) -> np.ndarray:
    from concourse.bass_utils import run_bass_kernel_spmd

    global _NC_CACHE
    if _NC_CACHE is None:
        _NC_CACHE = build()
    nc = _NC_CACHE

    audio = np.ascontiguousarray(audio_feats, dtype=np.float32)
    visual = np.ascontiguousarray(visual_feats, dtype=np.float32)
    in_maps = []
    for i in range(N_CORES):
        in_maps.append({
            "audio": audio[i * BL:(i + 1) * BL].reshape(AROWS, D),
            "visual": visual[i * BL:(i + 1) * BL].reshape(VROWS, D),
        })
    res = run_bass_kernel_spmd(nc, in_maps, core_ids=list(range(N_CORES)))
    val = res.results[0]["out"][0, 0]
    return np.asarray(val, dtype=np.float32)


if __name__ == "__main__":
    rng = np.random.default_rng(0)
    a = rng.standard_normal((B, NA, D)).astype(np.float32)
    v = rng.standard_normal((B, NV, D)).astype(np.float32)
    print(kernel(a, v))
